# revision 55
# baseline (speedup 1.0000x reference)
"""NeuroSAT message-passing kernel for 8 Trainium2 NeuronCores (Bass/Tile).

Strategy
--------
The dense adjacency factors as A = D_row @ B @ D_col with B binary, so B
streams from HBM in fp8 (1.0/0.0 are exact in e4m3) as the *moving* matmul
operand against bf16 stationary message tiles; the degree scalings are
per-partition activation scales / free tensor_tensor multiplies at PSUM
eviction, and the (scaling-entangled) final-layer MLP biases become rank-1
K=1 matmul corrections accumulated straight into the LSTM gate PSUMs.

Sharding (8 cores):
  - clauses: core k owns [2048k, 2048k+2048)
  - literals: core k owns [512k, 512k+512) u [4096+512k, 4096+512k+512)
    (a positive block and its negation block, so NeuroSAT's "flip" is a
    local slice swap instead of a cross-core exchange)
All row-wise ops (MLPs, LSTMs) run on the local shard in feature-major
layout [dim(128) x rows]; the two A-applications per round contract over
the full lit/clause axes, fed by AllGathers of the scaled row-major
L/C messages. Each AllGather is split in two and the contraction loops
are ordered chunk-major so collectives and B-streaming DMAs hide under
the previous chunk's matmuls (keeps TensorE warm through the round).
"""
import sys

sys.path.insert(0, "/opt/trn_rl_repo")

import numpy as np
import ml_dtypes

import concourse.bass as bass
import concourse.mybir as mybir
import concourse.tile as tile
from concourse import bacc
from concourse import bass_utils

dt = mybir.dt
AF = mybir.ActivationFunctionType
ALU = mybir.AluOpType
bf16 = ml_dtypes.bfloat16
f8 = ml_dtypes.float8_e4m3

NCORES = 8
D = 128
NL_TOT, NCL_TOT, NV = 8192, 16384, 4096
NL = NL_TOT // NCORES      # 1024 lits per core
NCL = NCL_TOT // NCORES    # 2048 clauses per core
KT1 = NL_TOT // 128        # 64  k-tiles for A^T @ Lpre
KT2 = NCL_TOT // 128       # 128 k-tiles for A @ Cpre
FP8_ONE = 0x38             # bit pattern of 1.0 in float8_e4m3
GAIN = np.float32(128.0)   # power-of-2 pre-scale keeping fp8 messages normal-range


# ---------------------------------------------------------------------------
# device program
# ---------------------------------------------------------------------------

def build_program(rounds: int):
    nc = bacc.Bacc("TRN2", target_bir_lowering=False, debug=False,
                   num_devices=NCORES)

    def inp(name, shape, dty):
        return nc.dram_tensor(name, list(shape), dty, kind="ExternalInput")

    # B streams, pre-tiled to match the chunk-major contraction loops:
    # b1[nn, h]: slab of 32 k-tiles [128p, 32tt, 512c] covering clause chunk
    #            nn, lit-tile phase h (tt = 4k + jj, global tile t = 8k+4h+jj)
    # b2[nn, h, s]: slab of 32 k-tiles [128p, 32tt, 512l] covering lit chunk
    #            nn, clause-tile phase h, half s (tt=8kk+j2, T=16(4s+kk)+8h+j2)
    b1 = inp("b1", [4, 2, 128, 32 * 512], dt.float8e4)
    b2 = inp("b2", [2, 2, 2, 128, 32 * 512], dt.float8e4)
    w = {}
    for p in ("lm", "cm", "lv"):
        for l in ("w1t", "w2t", "w3t"):
            shape = [128, 1] if (p, l) == ("lv", "w3t") else [128, 128]
            w[f"{p}_{l}"] = inp(f"{p}_{l}", shape, dt.bfloat16)
        for l in ("b1", "b2"):
            w[f"{p}_{l}"] = inp(f"{p}_{l}", [128, 1], dt.float32)
    cu_wt = inp("cu_wt", [128, 512], dt.bfloat16)      # cu_wih.T
    cu_ut = inp("cu_ut", [128, 512], dt.bfloat16)      # cu_whh.T
    cu_b = inp("cu_b", [128, 4], dt.float32)
    lu_wcl = inp("lu_wcl", [128, 512], dt.bfloat16)    # lu_wih[:, :128].T
    lu_wfl = inp("lu_wfl", [128, 512], dt.bfloat16)    # lu_wih[:, 128:].T
    lu_ut = inp("lu_ut", [128, 512], dt.bfloat16)      # lu_whh.T
    lu_b = inp("lu_b", [128, 4], dt.float32)
    lm_b3r = inp("lm_b3r", [1, 512], dt.bfloat16)      # lm_b3 tiled 4x
    cm_b3r = inp("cm_b3r", [1, 512], dt.bfloat16)      # cm_b3 tiled 4x
    ones1 = inp("ones1", [1, 128], dt.bfloat16)
    # round-0 shortcut: Lh0 is a broadcast vector, so round 0's dir-1 output
    # is rank-1: ps1 = u0 (x) sb1r with u0 = mlp(lh0)+b3 (host, exact) and
    # sb1r = GAIN * (B^T row) over my clauses
    u0 = inp("u0", [1, 128], dt.bfloat16)
    sb1r = inp("sb1r", [1, NCL], dt.bfloat16)
    colb = inp("colb", [128, NCL], dt.bfloat16)         # col bcast over partitions
    rowb = inp("rowb", [128, NL], dt.bfloat16)          # row bcast over partitions
    rowsc = inp("rowsc", [128, 8], dt.float32)         # row, per lit-tile column
    colsc = inp("colsc", [128, 16], dt.float32)        # col, per clause-tile column
    lh0 = inp("lh0", [128, NL], dt.bfloat16)
    ch0 = inp("ch0", [128, NCL], dt.bfloat16)

    vote_out = nc.dram_tensor("vote", [1, NL], dt.float32, kind="ExternalOutput")
    # per-phase AllGather buffers: AG1a/b carry lit chunks 0/1 (the tiles
    # dir-1 phase h=0/1 contracts); AG2a/b carry clause chunks {0,1}/{2,3}
    # (what dir-2 phase h=0/1 contracts).
    ag1_out = [nc.dram_tensor(f"ag1{h}_out", [NCORES, 128, 512], dt.float8e4,
                              addr_space="Shared") for h in range(2)]
    ag2_out = [nc.dram_tensor(f"ag2{q}_out", [NCORES, 128, 512], dt.float8e4,
                              addr_space="Shared") for q in range(4)]
    rg = [list(range(NCORES))]

    with tile.TileContext(nc) as tc:
        with (
            tc.tile_pool(name="const", bufs=1) as cp,
            tc.tile_pool(name="state", bufs=1) as sp,
            tc.tile_pool(name="work", bufs=1) as wp,
            tc.tile_pool(name="chunk", bufs=2) as kp,
            tc.tile_pool(name="bstream", bufs=4) as bp,
            tc.tile_pool(name="psd", bufs=1, space="PSUM") as psd,
            tc.tile_pool(name="psg", bufs=1, space="PSUM") as psg,
            tc.tile_pool(name="psm", bufs=2, space="PSUM") as psm,
            tc.tile_pool(name="dram", bufs=1, space="DRAM") as dp,
        ):
            # ---- constants into SBUF ----
            C = {}
            for name, t in [
                ("cu_wt", cu_wt), ("cu_ut", cu_ut), ("lu_wcl", lu_wcl),
                ("lu_wfl", lu_wfl), ("lu_ut", lu_ut),
            ]:
                C[name] = cp.tile([128, 512], dt.bfloat16, name=name)
                nc.scalar.dma_start(out=C[name], in_=t.ap())
            for p in ("lm", "cm", "lv"):
                for l in ("w1t", "w2t", "w3t"):
                    shape = [128, 1] if (p, l) == ("lv", "w3t") else [128, 128]
                    C[f"{p}_{l}"] = cp.tile(shape, dt.bfloat16, name=f"{p}_{l}")
                    nc.scalar.dma_start(out=C[f"{p}_{l}"], in_=w[f"{p}_{l}"].ap())
                for l in ("b1", "b2"):
                    C[f"{p}_{l}"] = cp.tile([128, 1], dt.float32, name=f"{p}_{l}")
                    nc.scalar.dma_start(out=C[f"{p}_{l}"], in_=w[f"{p}_{l}"].ap())
            for name, t, shape, dty in [
                ("cu_b", cu_b, [128, 4], dt.float32),
                ("lu_b", lu_b, [128, 4], dt.float32),
                ("lm_b3r", lm_b3r, [1, 512], dt.bfloat16),
                ("cm_b3r", cm_b3r, [1, 512], dt.bfloat16),
                ("ones1", ones1, [1, 128], dt.bfloat16),
                ("u0", u0, [1, 128], dt.bfloat16),
                ("sb1r", sb1r, [1, NCL], dt.bfloat16),
                ("colb", colb, [128, NCL], dt.bfloat16),
                ("rowb", rowb, [128, NL], dt.bfloat16),
                ("rowsc", rowsc, [128, 8], dt.float32),
                ("colsc", colsc, [128, 16], dt.float32),
            ]:
                C[name] = cp.tile(shape, dty, name=name)
                nc.scalar.dma_start(out=C[name], in_=t.ap())

            # ---- states ----
            Lh_pp = [sp.tile([128, NL], dt.bfloat16, name="Lh_a"),
                     sp.tile([128, NL], dt.bfloat16, name="Lh_b")]
            Ch = sp.tile([128, NCL], dt.bfloat16, name="Ch")
            Lc = sp.tile([128, NL], dt.bfloat16, name="Lc")
            Cc = sp.tile([128, NCL], dt.bfloat16, name="Cc")
            nc.scalar.dma_start(out=Lh_pp[0], in_=lh0.ap())
            nc.scalar.dma_start(out=Ch, in_=ch0.ap())
            nc.vector.memset(Lc, 0.0)
            nc.vector.memset(Cc, 0.0)

            # ---- resident slices of B: clause-chunk 0 of b1 (both phases)
            # plus a few pinned b2 slab-halves (cuts per-round streaming) ----
            b1res = [cp.tile([128, 32 * 512], dt.float8e4, name=f"b1res{h}")
                     for h in range(2)]
            for h in range(2):
                nc.sync.dma_start(out=b1res[h], in_=b1.ap()[0, h])
            PIN = [(0, 0, 0, 0), (0, 1, 0, 0), (1, 0, 0, 0), (1, 1, 0, 0), (0, 0, 1, 0)]
            b2res = {}
            for (pn, ph, psl, ps2) in PIN:
                tpin = cp.tile([128, 16 * 512], dt.float8e4,
                               name=f"b2res{pn}{ph}{psl}{ps2}")
                nc.sync.dma_start(
                    out=tpin,
                    in_=b2.ap()[pn, ph, psl][:, 16 * 512 * ps2:
                                             16 * 512 * (ps2 + 1)])
                b2res[(pn, ph, psl, ps2)] = tpin

            # ---- round-persistent work tiles ----
            lpre_img = wp.tile([128, NL], dt.float8e4, name="lpre_img")
            cpre_img = wp.tile([128, NCL], dt.float8e4, name="cpre_img")
            # gathered message halves: lpre_half[h] holds, for every core kk,
            # its chunk-h message tiles (4 tiles of 128 lits each) at columns
            # [512*kk, 512*kk+512); cpre_half[h] likewise with 8 tiles of 128
            # clauses at [1024*kk, 1024*kk+1024).
            lpre_half = [wp.tile([128, 8 * 512], dt.float8e4, name=f"lpre_h{h}")
                         for h in range(2)]
            cpre_half = [wp.tile([128, 8 * 1024], dt.float8e4, name=f"cpre_h{h}")
                         for h in range(2)]
            ag1_in = [dp.tile([128, 512], dt.float8e4, name=f"ag1{h}_in")
                      for h in range(2)]
            ag2_in = [dp.tile([128, 512], dt.float8e4, name=f"ag2{q}_in")
                      for q in range(4)]

            def mlp3_chunk(src, w1t, b1_, w2t, b2_, w3t, b3r, img, nn, sc):
                """3-layer MLP on one 512-col chunk; the 3rd layer transposes
                tile-wise into `img` and folds b3 in as a rank-1 matmul so the
                fp8 messages carry the full affine output (no gate-side
                correction needed)."""
                tag_sfx = "m"
                h1 = kp.tile([128, 512], dt.bfloat16, tag="mh1", bufs=2,
                             name=f"mh1_{nn}")
                h2 = kp.tile([128, 512], dt.bfloat16, tag="mh2", bufs=2,
                             name=f"mh2_{nn}")
                ps = psm.tile([128, 512], dt.float32, tag=tag_sfx,
                              name=f"mm1_{nn}")
                nc.tensor.matmul(ps, w1t, src, start=True, stop=True)
                nc.scalar.activation(h1, ps, AF.Relu, bias=b1_)
                ps = psm.tile([128, 512], dt.float32, tag=tag_sfx,
                              name=f"mm2_{nn}")
                nc.tensor.matmul(ps, w2t, h1, start=True, stop=True)
                nc.scalar.activation(h2, ps, AF.Relu, bias=b2_)
                ps3 = psm.tile([128, 512], dt.float32, tag=tag_sfx,
                               name=f"mm3_{nn}")
                for jj in range(4):
                    nc.tensor.matmul(ps3[:, 128 * jj:128 * (jj + 1)],
                                     h2[:, 128 * jj:128 * (jj + 1)], w3t,
                                     start=(jj == 0), stop=False,
                                     skip_group_check=True)
                nc.tensor.matmul(ps3, C["ones1"], b3r,
                                 start=False, stop=True, skip_group_check=True)
                for jj in range(4):
                    j = 4 * nn + jj
                    nc.scalar.activation(img[:, 128 * j:128 * (j + 1)],
                                         ps3[:, 128 * jj:128 * (jj + 1)],
                                         AF.Copy, scale=sc[:, j:j + 1])

            def l_msg_chunk(Lh_src, nn):
                """L-message for lit chunk nn (512 lits) + AG1-half kickoff."""
                sl = slice(512 * nn, 512 * (nn + 1))
                mlp3_chunk(Lh_src[:, sl], C["lm_w1t"], C["lm_b1"],
                           C["lm_w2t"], C["lm_b2"], C["lm_w3t"], C["lm_b3r"],
                           lpre_img, nn, C["rowsc"])
                nc.gpsimd.dma_start(out=ag1_in[nn], in_=lpre_img[:, sl])
                nc.gpsimd.collective_compute(
                    "AllGather", ALU.bypass, replica_groups=rg,
                    ins=[ag1_in[nn].opt()], outs=[ag1_out[nn].ap().opt()])

            def land_ag1(h):
                for kk in range(0, NCORES, 2):
                    nc.gpsimd.dma_start(
                        out=lpre_half[h][:, 512 * kk:512 * (kk + 2)]
                        .rearrange("p (k c) -> p k c", k=2),
                        in_=ag1_out[h].ap()[kk:kk + 2]
                        .rearrange("k p c -> p k c"))

            def c_msg_chunk(nn):
                """C-message for clause chunk nn (512 clauses)."""
                sl = slice(512 * nn, 512 * (nn + 1))
                mlp3_chunk(Ch[:, sl], C["cm_w1t"], C["cm_b1"],
                           C["cm_w2t"], C["cm_b2"], C["cm_w3t"], C["cm_b3r"],
                           cpre_img, nn, C["colsc"])
                nc.gpsimd.dma_start(out=ag2_in[nn], in_=cpre_img[:, sl])
                nc.gpsimd.collective_compute(
                    "AllGather", ALU.bypass, replica_groups=rg,
                    ins=[ag2_in[nn].opt()], outs=[ag2_out[nn].ap().opt()])

            def land_ag2(h):
                # phase h consumes clause chunks {2h, 2h+1}: quarter-gather q
                # lands at columns 1024*kk + 512*(q%2)
                for q in (2 * h, 2 * h + 1):
                    for kk in range(NCORES):
                        nc.gpsimd.dma_start(
                            out=cpre_half[h][:, 1024 * kk + 512 * (q % 2):
                                             1024 * kk + 512 * (q % 2) + 512],
                            in_=ag2_out[q].ap()[kk])

            def lstm_chunk(which, cc, xin, Lh_src=None, Lh_dst=None):
                """LSTM gate + state update for one 512-col chunk."""
                sl = slice(512 * cc, 512 * (cc + 1))
                gts = []
                for g in range(4):
                    gs = slice(128 * g, 128 * (g + 1))
                    ps = psg.tile([128, 512], dt.float32, tag=f"g{g % 2}",
                                  name=f"ps_{which}_{cc}_{g}")
                    if which == "c":
                        nc.tensor.matmul(ps, C["cu_wt"][:, gs], xin,
                                         start=True, stop=False,
                                         skip_group_check=True)
                        nc.tensor.matmul(ps, C["cu_ut"][:, gs], Ch[:, sl],
                                         start=False, stop=True,
                                         skip_group_check=True)
                        bias = C["cu_b"][:, g:g + 1]
                    else:
                        flip_sl = slice(512 * (1 - cc), 512 * (2 - cc))
                        nc.tensor.matmul(ps, C["lu_wcl"][:, gs], xin,
                                         start=True, stop=False,
                                         skip_group_check=True)
                        nc.tensor.matmul(ps, C["lu_wfl"][:, gs],
                                         Lh_src[:, flip_sl],
                                         start=False, stop=False,
                                         skip_group_check=True)
                        nc.tensor.matmul(ps, C["lu_ut"][:, gs], Lh_src[:, sl],
                                         start=False, stop=True,
                                         skip_group_check=True)
                        bias = C["lu_b"][:, g:g + 1]
                    gt = kp.tile([128, 512], dt.bfloat16, tag=f"gate{g}",
                                 bufs=1, name=f"gt_{which}_{cc}_{g}")
                    nc.scalar.activation(gt, ps,
                                         AF.Tanh if g == 2 else AF.Sigmoid,
                                         bias=bias)
                    gts.append(gt)
                cell = Cc if which == "c" else Lc
                hout = Ch if which == "c" else Lh_dst
                t1 = kp.tile([128, 512], dt.bfloat16, tag="t1", bufs=1,
                             name=f"t1_{which}_{cc}")
                t2 = kp.tile([128, 512], dt.bfloat16, tag="t2", bufs=1,
                             name=f"t2_{which}_{cc}")
                nc.vector.tensor_tensor(out=t1, in0=gts[1], in1=cell[:, sl],
                                        op=ALU.mult)
                nc.vector.tensor_tensor(out=t2, in0=gts[0], in1=gts[2],
                                        op=ALU.mult)
                nc.vector.tensor_tensor(out=cell[:, sl], in0=t1, in1=t2,
                                        op=ALU.add)
                t3 = kp.tile([128, 512], dt.bfloat16, tag="t3", bufs=1,
                             name=f"t3_{which}_{cc}")
                nc.scalar.activation(t3, cell[:, sl], AF.Tanh)
                nc.vector.tensor_tensor(out=hout[:, sl], in0=gts[3], in1=t3,
                                        op=ALU.mult)

            # Manual phase pinning: monotonically increasing scheduler-sim
            # timestamps force the emitted per-engine instruction order to
            # follow the hand-pipelined phase order. Without this, the
            # scheduler's naive collective cost model emits AG-completion
            # waits (landing copies) ahead of the next collective's trigger
            # on the gpsimd queue, head-blocking it for ~15us per round.
            _ph = [0]

            def phase():
                _ph[0] += 1
                return tc.tile_wait_until(_ph[0])

            # (no prologue: round 0's dir-1 collapses to rank-1 matmuls, so
            # no round-0 L messages or AG1s are needed)

            for r in range(rounds):
                Lh = Lh_pp[r % 2]
                Lh_new = Lh_pp[(r + 1) % 2]

                # ===== dir-1, group-major: clause chunks {0,1} are fully
                # contracted (h=0 then h=1) and their C side run first, so
                # AG2a kicks at ~50% of the C-phase with the whole second
                # group as its in-flight cover; group {2,3} then feeds AG2b,
                # which flies over dir-2 h=0 =====
                ps1 = [psd.tile([128, 512], dt.float32, tag=f"d{nn}",
                                name=f"ps1_{r}_{nn}") for nn in range(4)]

                def d1_contract(nn, h):
                    for s2 in range(2):
                        if nn == 0:
                            b1t = b1res[h][:, 16 * 512 * s2:
                                           16 * 512 * (s2 + 1)]
                        else:
                            b1t = bp.tile([128, 16 * 512], dt.float8e4,
                                          tag="b1", bufs=3,
                                          name=f"b1_{r}_{nn}_{h}_{s2}")
                            nc.sync.dma_start(
                                out=b1t,
                                in_=b1.ap()[nn, h][:, 16 * 512 * s2:
                                                   16 * 512 * (s2 + 1)])
                        for ttp in range(8):
                            tt = 16 * s2 + 2 * ttp
                            lhsT = lpre_half[h][:, 128 * tt:128 * (tt + 2)] \
                                .rearrange("p (e d) -> p e d", e=2)
                            rhs = b1t[:, 1024 * ttp:1024 * (ttp + 1)] \
                                .rearrange("p (e c) -> p e c", e=2)
                            nc.tensor.matmul(
                                ps1[nn], lhsT, rhs,
                                start=(h == 0 and tt == 0),
                                stop=(h == 1 and tt == 30),
                                perf_mode=mybir.MatmulPerfMode.DoubleRow,
                                skip_group_check=True)

                def c_xin(cn):
                    # hoisted ahead of the LSTM chains so the vector queue
                    # never head-blocks a gate matmul on a late xin
                    sl = slice(512 * cn, 512 * (cn + 1))
                    xin = kp.tile([128, 512], dt.bfloat16, tag="xin",
                                  bufs=2, name=f"lcs_{r}_{cn}")
                    nc.vector.tensor_tensor(out=xin, in0=ps1[cn],
                                            in1=C["colb"][:, sl],
                                            op=ALU.mult)
                    return xin

                def c_rest(cn, xin):
                    lstm_chunk("c", cn, xin)
                    c_msg_chunk(cn)

                for g in range(2):
                    n0, n1 = 2 * g, 2 * g + 1
                    if r == 0:
                        with phase():
                            for cn in (n0, n1):
                                sl = slice(512 * cn, 512 * (cn + 1))
                                nc.tensor.matmul(ps1[cn], C["u0"],
                                                 C["sb1r"][0:1, sl],
                                                 start=True, stop=True,
                                                 skip_group_check=True)
                            x0 = c_xin(n0)
                            x1 = c_xin(n1)
                            c_rest(n0, x0)
                            c_rest(n1, x1)      # kicks AG2a / AG2b
                        if g == 0:
                            with phase():
                                land_ag2(0)
                    elif g == 0:
                        with phase():
                            land_ag1(0)
                        with phase():
                            d1_contract(n0, 0)
                            d1_contract(n1, 0)
                        with phase():
                            land_ag1(1)
                        with phase():
                            d1_contract(n0, 1)
                            x0 = c_xin(n0)
                            d1_contract(n1, 1)  # c_rest(n0) hides under this
                            x1 = c_xin(n1)
                            c_rest(n0, x0)
                            c_rest(n1, x1)      # kicks AG2a
                        with phase():
                            land_ag2(0)         # q0/q1 land under g1 compute
                    else:
                        with phase():
                            d1_contract(n0, 0)
                            d1_contract(n1, 0)
                            d1_contract(n0, 1)
                            x0 = c_xin(n0)
                            d1_contract(n1, 1)
                            x1 = c_xin(n1)
                            c_rest(n0, x0)
                            c_rest(n1, x1)      # kicks AG2b

                # ===== dir-2, phase-major: h=0 contracts the AG2a tiles for
                # both lit chunks while AG2b flies; h=1 completes each chunk
                # and runs the L side, kicking the next round's AG1 halves =====
                ps2 = [psd.tile([128, 512], dt.float32, tag=f"d{nn}",
                                name=f"ps2_{r}_{nn}") for nn in range(2)]
                lxin = [None, None]
                for h in range(2):
                    if h == 1:
                        with phase():
                            land_ag2(1)
                    with phase():
                        for nn in range(3 if h == 1 else 2):
                            if nn < 2:
                                for s in range(2):
                                    for s2 in range(2):
                                        if (nn, h, s, s2) in b2res:
                                            b2t = b2res[(nn, h, s, s2)]
                                        else:
                                            b2t = bp.tile([128, 16 * 512],
                                                          dt.float8e4,
                                                          tag="b2", bufs=4,
                                                          name=f"b2_{r}_{nn}_{h}_{s}_{s2}")
                                            nc.sync.dma_start(
                                                out=b2t,
                                                in_=b2.ap()[nn, h, s][:, 16 * 512 * s2:
                                                                      16 * 512 * (s2 + 1)])
                                        for ttp in range(8):
                                            tt = 16 * s2 + 2 * ttp
                                            lhsT = cpre_half[h][:, 4096 * s + 128 * tt:
                                                                4096 * s + 128 * (tt + 2)] \
                                                .rearrange("p (e d) -> p e d", e=2)
                                            rhs = b2t[:, 1024 * ttp:1024 * (ttp + 1)] \
                                                .rearrange("p (e c) -> p e c", e=2)
                                            nc.tensor.matmul(
                                                ps2[nn], lhsT, rhs,
                                                start=(h == 0 and s == 0 and tt == 0),
                                                stop=(h == 1 and s == 1 and tt == 30),
                                                perf_mode=mybir.MatmulPerfMode.DoubleRow,
                                                skip_group_check=True)
                            if h == 1 and nn < 2:
                                sl = slice(512 * nn, 512 * (nn + 1))
                                lxin[nn] = kp.tile([128, 512], dt.bfloat16,
                                                   tag="xin", bufs=2,
                                                   name=f"cls_{r}_{nn}")
                                nc.vector.tensor_tensor(out=lxin[nn],
                                                        in0=ps2[nn],
                                                        in1=C["rowb"][:, sl],
                                                        op=ALU.mult)
                            if h == 1 and nn > 0:
                                cn = nn - 1
                                lstm_chunk("l", cn, lxin[cn], Lh_src=Lh,
                                           Lh_dst=Lh_new)
                                if r < rounds - 1:
                                    l_msg_chunk(Lh_new, cn)

            # ===== vote MLP (bias of last layer added host-side) =====
            Lh_fin = Lh_pp[rounds % 2]
            vote_sb = wp.tile([1, NL], dt.float32, name="vote_sb")
            for nn in range(2):
                sl = slice(512 * nn, 512 * (nn + 1))
                vh1 = kp.tile([128, 512], dt.bfloat16, tag="mh1", bufs=2,
                              name=f"vh1_{nn}")
                vh2 = kp.tile([128, 512], dt.bfloat16, tag="mh2", bufs=2,
                              name=f"vh2_{nn}")
                ps = psm.tile([128, 512], dt.float32, tag="m", name=f"v1_{nn}")
                nc.tensor.matmul(ps, C["lv_w1t"], Lh_fin[:, sl],
                                 start=True, stop=True)
                nc.scalar.activation(vh1, ps, AF.Relu, bias=C["lv_b1"])
                ps = psm.tile([128, 512], dt.float32, tag="m", name=f"v2_{nn}")
                nc.tensor.matmul(ps, C["lv_w2t"], vh1,
                                 start=True, stop=True)
                nc.scalar.activation(vh2, ps, AF.Relu, bias=C["lv_b2"])
                ps = psm.tile([1, 512], dt.float32, tag="m", name=f"v3_{nn}")
                nc.tensor.matmul(ps, C["lv_w3t"], vh2,
                                 start=True, stop=True)
                nc.scalar.activation(vote_sb[0:1, sl], ps, AF.Copy)
            nc.scalar.dma_start(out=vote_out.ap(), in_=vote_sb)

    nc.compile()
    return nc


# ---------------------------------------------------------------------------
# host-side input preparation
# ---------------------------------------------------------------------------

def prep_inputs(inputs):
    g = {k: np.asarray(v) for k, v in inputs.items()}
    lit_idx = g["lit_idx"].astype(np.int64)
    clause_idx = g["clause_idx"].astype(np.int64)

    B = np.zeros((NL_TOT, NCL_TOT), np.bool_)
    B[lit_idx, clause_idx] = True
    degc = B.sum(0).astype(np.float32)
    degl = B.sum(1).astype(np.float32)
    col = (np.float32(1.0) / (np.sqrt(degc) + np.float32(1e-6))).astype(np.float32)
    row = (np.float32(1.0) / (np.sqrt(degl) + np.float32(1e-6))).astype(np.float32)
    # degree-0 rows/cols of A are structurally zero: clamp their scales so the
    # gained fp8 messages stay finite (mathematically identical result)
    col = np.where(degc > 0, col, np.float32(0)).astype(np.float32)
    row = np.where(degl > 0, row, np.float32(0)).astype(np.float32)

    # permuted lit order: core k <- [512k..512k+512) u [4096+512k..4096+512k+512)
    lit_order = np.concatenate(
        [np.concatenate([np.arange(512 * k, 512 * (k + 1)),
                         NV + np.arange(512 * k, 512 * (k + 1))])
         for k in range(NCORES)])
    Bu = B.astype(np.uint8) * FP8_ONE
    Bp = Bu[lit_order]                      # [8192, 16384] permuted rows
    row_p = row[lit_order]

    def b(x):
        return np.ascontiguousarray(np.asarray(x, np.float32)).astype(bf16)

    # round-0 shortcut constants: u0 = lm_mlp(lh0_vec)+lm_b3 (exact, host),
    # sb1 = GAIN * (row @ B) per clause
    lh0_vec = np.asarray(g["L_init_w"], np.float32)[:, 0] + np.asarray(
        g["L_init_b"], np.float32)
    _h = np.maximum(np.asarray(g["lm_w1"], np.float32) @ lh0_vec
                    + np.asarray(g["lm_b1"], np.float32), 0)
    _h = np.maximum(np.asarray(g["lm_w2"], np.float32) @ _h
                    + np.asarray(g["lm_b2"], np.float32), 0)
    u0_vec = np.asarray(g["lm_w3"], np.float32) @ _h + np.asarray(
        g["lm_b3"], np.float32)
    sb1_full = GAIN * (row @ B.astype(np.float32))

    common = {
        "lm_w1t": b(g["lm_w1"].T), "lm_w2t": b(g["lm_w2"].T), "lm_w3t": b(g["lm_w3"].T),
        "cm_w1t": b(g["cm_w1"].T), "cm_w2t": b(g["cm_w2"].T), "cm_w3t": b(g["cm_w3"].T),
        "lv_w1t": b(g["lv_w1"].T), "lv_w2t": b(g["lv_w2"].T), "lv_w3t": b(g["lv_w3"].T),
        "lm_b1": np.asarray(g["lm_b1"], np.float32).reshape(128, 1),
        "lm_b2": np.asarray(g["lm_b2"], np.float32).reshape(128, 1),
        "cm_b1": np.asarray(g["cm_b1"], np.float32).reshape(128, 1),
        "cm_b2": np.asarray(g["cm_b2"], np.float32).reshape(128, 1),
        "lv_b1": np.asarray(g["lv_b1"], np.float32).reshape(128, 1),
        "lv_b2": np.asarray(g["lv_b2"], np.float32).reshape(128, 1),
        "cu_wt": b(g["cu_wih"].T), "cu_ut": b(g["cu_whh"].T),
        "lu_wcl": b(g["lu_wih"][:, :D].T), "lu_wfl": b(g["lu_wih"][:, D:].T),
        "lu_ut": b(g["lu_whh"].T),
        "cu_b": np.asarray(g["cu_bih"] + g["cu_bhh"], np.float32).reshape(4, 128).T.copy(),
        "lu_b": np.asarray(g["lu_bih"] + g["lu_bhh"], np.float32).reshape(4, 128).T.copy(),
        "lm_b3r": b(np.tile(np.asarray(g["lm_b3"], np.float32), 4)).reshape(1, 512),
        "cm_b3r": b(np.tile(np.asarray(g["cm_b3"], np.float32), 4)).reshape(1, 512),
        "ones1": np.ones((1, 128), dtype=bf16),
        "u0": b(u0_vec).reshape(1, 128),
        "lh0": np.ascontiguousarray(np.broadcast_to(
            np.asarray(g["L_init_w"][:, 0] + g["L_init_b"], np.float32)[:, None],
            (128, NL))).astype(bf16),
        "ch0": np.ascontiguousarray(np.broadcast_to(
            np.asarray(g["C_init_w"][:, 0] + g["C_init_b"], np.float32)[:, None],
            (128, NCL))).astype(bf16),
    }

    in_maps = []
    for k in range(NCORES):
        lsl = slice(NL * k, NL * (k + 1))
        csl = slice(NCL * k, NCL * (k + 1))
        # b1: Bp[:, csl] is [t(64)*128p rows, nn(4)*512c cols]
        #     [kk(8), jh(2), jj(4), p, nn, c] -> [nn, jh, p, kk, jj, c]
        X = Bp[:, csl].reshape(8, 2, 4, 128, 4, 512)
        b1k = np.ascontiguousarray(X.transpose(4, 1, 3, 0, 2, 5)).reshape(
            4, 2, 128, 32 * 512).view(f8)
        # b2: Bp[lsl, :].T is [T(128)*128p rows, nn(2)*512l cols]
        #     [ks(2), kk(4), h(2), j2(8), p, nn, l] -> [nn, h, ks, p, kk, j2, l]
        Y = Bp[lsl, :].T.reshape(2, 4, 2, 8, 128, 2, 512)
        b2k = np.ascontiguousarray(Y.transpose(5, 2, 0, 4, 1, 3, 6)).reshape(
            2, 2, 2, 128, 32 * 512).view(f8)
        m = dict(common)
        m.update({
            "b1": b1k,
            "b2": b2k,
            "sb1r": sb1_full[csl].astype(bf16).reshape(1, NCL),
            "colb": np.ascontiguousarray(
                np.broadcast_to(col[csl][None, :] / GAIN, (128, NCL))).astype(bf16),
            "rowb": np.ascontiguousarray(
                np.broadcast_to(row_p[lsl][None, :] / GAIN, (128, NL))).astype(bf16),
            "rowsc": np.ascontiguousarray(
                GAIN * row_p[lsl].reshape(8, 128).T).astype(np.float32),
            "colsc": np.ascontiguousarray(
                GAIN * col[csl].reshape(16, 128).T).astype(np.float32),
        })
        in_maps.append(m)
    return in_maps


def selfcheck_layouts(in_maps, lit_idx, clause_idx):
    """Random probes: device-layout b1/b2 entries vs the raw B matrix."""
    B = np.zeros((NL_TOT, NCL_TOT), np.uint8)
    B[lit_idx, clause_idx] = FP8_ONE
    lit_order = np.concatenate(
        [np.concatenate([np.arange(512 * k, 512 * (k + 1)),
                         NV + np.arange(512 * k, 512 * (k + 1))])
         for k in range(NCORES)])
    Bp = B[lit_order]
    rng = np.random.default_rng(1)
    for k in (0, 3):
        b1k = in_maps[k]["b1"].view(np.uint8).reshape(4, 2, 128, 32, 512)
        for _ in range(50):
            nn, h, p, tt, c = (rng.integers(4), rng.integers(2), rng.integers(128),
                               rng.integers(32), rng.integers(512))
            t = 8 * (tt // 4) + 4 * h + tt % 4
            want = Bp[128 * t + p, 2048 * k + 512 * nn + c]
            assert b1k[nn, h, p, tt, c] == want, (k, nn, h, p, tt, c)
        b2k = in_maps[k]["b2"].view(np.uint8).reshape(2, 2, 2, 128, 32, 512)
        for _ in range(50):
            nn, h, s, p, tt, c = (rng.integers(2), rng.integers(2), rng.integers(2),
                                  rng.integers(128), rng.integers(32), rng.integers(512))
            T = 16 * (4 * s + tt // 8) + 8 * h + tt % 8
            want = Bp[1024 * k + 512 * nn + c, 128 * T + p]
            assert b2k[nn, h, s, p, tt, c] == want, (k, nn, h, s, p, tt, c)


_PROGRAM_CACHE = {}


def _get_program(rounds):
    if rounds not in _PROGRAM_CACHE:
        _PROGRAM_CACHE[rounds] = build_program(rounds)
    return _PROGRAM_CACHE[rounds]


def run_device(inputs, trace=False, rounds=None, **kw):
    if rounds is None:
        rounds = int(inputs.get("n_rounds", 16))
    in_maps = prep_inputs(inputs)
    nc = _get_program(rounds)
    res = bass_utils.run_bass_kernel_spmd(
        nc, in_maps, core_ids=list(range(NCORES)), trace=trace, **kw)
    return res


def assemble_votes(res_results, lv_b3):
    votes = np.stack([np.asarray(res_results[k]["vote"]).reshape(NL)
                      for k in range(NCORES)])   # [8, 1024]
    vote = votes + np.float32(lv_b3)
    pos = vote[:, :512].reshape(NV)              # var v -> core v//512
    neg = vote[:, 512:].reshape(NV)
    vj = np.stack([pos, neg], axis=1)            # [4096, 2]
    return vj.reshape(32, -1).mean(axis=1).astype(np.float32)


def kernel(**inputs) -> np.ndarray:
    res = run_device(inputs)
    return assemble_votes(res.results, np.asarray(inputs["lv_b3"]).reshape(-1)[0])



# revision 57
# speedup vs baseline: 1.0426x; 1.0426x over previous
"""NeuroSAT message-passing kernel for 8 Trainium2 NeuronCores (Bass/Tile).

Strategy
--------
The dense adjacency factors as A = D_row @ B @ D_col with B binary, so B
streams from HBM in fp8 (1.0/0.0 are exact in e4m3) as the *moving* matmul
operand against bf16 stationary message tiles; the degree scalings are
per-partition activation scales / free tensor_tensor multiplies at PSUM
eviction, and the (scaling-entangled) final-layer MLP biases become rank-1
K=1 matmul corrections accumulated straight into the LSTM gate PSUMs.

Sharding (8 cores):
  - clauses: core k owns [2048k, 2048k+2048)
  - literals: core k owns [512k, 512k+512) u [4096+512k, 4096+512k+512)
    (a positive block and its negation block, so NeuroSAT's "flip" is a
    local slice swap instead of a cross-core exchange)
All row-wise ops (MLPs, LSTMs) run on the local shard in feature-major
layout [dim(128) x rows]; the two A-applications per round contract over
the full lit/clause axes, fed by AllGathers of the scaled row-major
L/C messages. Each AllGather is split in two and the contraction loops
are ordered chunk-major so collectives and B-streaming DMAs hide under
the previous chunk's matmuls (keeps TensorE warm through the round).
"""
import sys

sys.path.insert(0, "/opt/trn_rl_repo")

import numpy as np
import ml_dtypes

import concourse.bass as bass
import concourse.mybir as mybir
import concourse.tile as tile
from concourse import bacc
from concourse import bass_utils

dt = mybir.dt
AF = mybir.ActivationFunctionType
ALU = mybir.AluOpType
bf16 = ml_dtypes.bfloat16
f8 = ml_dtypes.float8_e4m3

NCORES = 8
D = 128
NL_TOT, NCL_TOT, NV = 8192, 16384, 4096
NL = NL_TOT // NCORES      # 1024 lits per core
NCL = NCL_TOT // NCORES    # 2048 clauses per core
KT1 = NL_TOT // 128        # 64  k-tiles for A^T @ Lpre
KT2 = NCL_TOT // 128       # 128 k-tiles for A @ Cpre
FP8_ONE = 0x38             # bit pattern of 1.0 in float8_e4m3
GAIN = np.float32(128.0)   # power-of-2 pre-scale keeping fp8 messages normal-range


# ---------------------------------------------------------------------------
# device program
# ---------------------------------------------------------------------------

def build_program(rounds: int):
    nc = bacc.Bacc("TRN2", target_bir_lowering=False, debug=False,
                   num_devices=NCORES)

    def inp(name, shape, dty):
        return nc.dram_tensor(name, list(shape), dty, kind="ExternalInput")

    # B streams, pre-tiled to match the chunk-major contraction loops:
    # b1[nn, h]: slab of 32 k-tiles [128p, 32tt, 512c] covering clause chunk
    #            nn, lit-tile phase h (tt = 4k + jj, global tile t = 8k+4h+jj)
    # b2[nn, h, s]: slab of 32 k-tiles [128p, 32tt, 512l] covering lit chunk
    #            nn, clause-tile phase h, half s (tt=8kk+j2, T=16(4s+kk)+8h+j2)
    b1 = inp("b1", [4, 2, 128, 32 * 512], dt.float8e4)
    b2 = inp("b2", [2, 2, 2, 128, 32 * 512], dt.float8e4)
    w = {}
    for p in ("lm", "cm", "lv"):
        for l in ("w1t", "w2t", "w3t"):
            shape = [128, 1] if (p, l) == ("lv", "w3t") else [128, 128]
            w[f"{p}_{l}"] = inp(f"{p}_{l}", shape, dt.bfloat16)
        for l in ("b1", "b2"):
            w[f"{p}_{l}"] = inp(f"{p}_{l}", [128, 1], dt.float32)
    cu_wt = inp("cu_wt", [128, 512], dt.bfloat16)      # cu_wih.T
    cu_ut = inp("cu_ut", [128, 512], dt.bfloat16)      # cu_whh.T
    cu_b = inp("cu_b", [128, 4], dt.float32)
    lu_wcl = inp("lu_wcl", [128, 512], dt.bfloat16)    # lu_wih[:, :128].T
    lu_wfl = inp("lu_wfl", [128, 512], dt.bfloat16)    # lu_wih[:, 128:].T
    lu_ut = inp("lu_ut", [128, 512], dt.bfloat16)      # lu_whh.T
    lu_b = inp("lu_b", [128, 4], dt.float32)
    lm_b3r = inp("lm_b3r", [1, 512], dt.bfloat16)      # lm_b3 tiled 4x
    cm_b3r = inp("cm_b3r", [1, 512], dt.bfloat16)      # cm_b3 tiled 4x
    ones1 = inp("ones1", [1, 128], dt.bfloat16)
    # round-0 shortcut: Lh0 is a broadcast vector, so round 0's dir-1 output
    # is rank-1: ps1 = u0 (x) sb1r with u0 = mlp(lh0)+b3 (host, exact) and
    # sb1r = GAIN * (B^T row) over my clauses
    u0 = inp("u0", [1, 128], dt.bfloat16)
    sb1r = inp("sb1r", [1, NCL], dt.bfloat16)
    colb = inp("colb", [128, NCL], dt.bfloat16)         # col bcast over partitions
    rowb = inp("rowb", [128, NL], dt.bfloat16)          # row bcast over partitions
    rowsc = inp("rowsc", [128, 8], dt.float32)         # row, per lit-tile column
    colsc = inp("colsc", [128, 16], dt.float32)        # col, per clause-tile column
    lh0 = inp("lh0", [128, NL], dt.bfloat16)
    ch0 = inp("ch0", [128, NCL], dt.bfloat16)

    vote_out = nc.dram_tensor("vote", [1, NL], dt.float32, kind="ExternalOutput")
    # per-phase AllGather buffers: AG1a/b carry lit chunks 0/1 (the tiles
    # dir-1 phase h=0/1 contracts); AG2a/b carry clause chunks {0,1}/{2,3}
    # (what dir-2 phase h=0/1 contracts).
    ag1_out = [nc.dram_tensor(f"ag1{h}_out", [NCORES, 128, 512], dt.float8e4,
                              addr_space="Shared") for h in range(2)]
    ag2_out = [nc.dram_tensor(f"ag2{q}_out", [NCORES, 128, 512], dt.float8e4,
                              addr_space="Shared") for q in range(4)]
    rg = [list(range(NCORES))]

    with tile.TileContext(nc) as tc:
        with (
            tc.tile_pool(name="const", bufs=1) as cp,
            tc.tile_pool(name="state", bufs=1) as sp,
            tc.tile_pool(name="work", bufs=1) as wp,
            tc.tile_pool(name="chunk", bufs=2) as kp,
            tc.tile_pool(name="bstream", bufs=4) as bp,
            tc.tile_pool(name="psd", bufs=1, space="PSUM") as psd,
            tc.tile_pool(name="psg", bufs=1, space="PSUM") as psg,
            tc.tile_pool(name="psm", bufs=2, space="PSUM") as psm,
            tc.tile_pool(name="dram", bufs=1, space="DRAM") as dp,
        ):
            # ---- constants into SBUF ----
            C = {}
            for name, t in [
                ("cu_wt", cu_wt), ("cu_ut", cu_ut), ("lu_wcl", lu_wcl),
                ("lu_wfl", lu_wfl), ("lu_ut", lu_ut),
            ]:
                C[name] = cp.tile([128, 512], dt.bfloat16, name=name)
                nc.scalar.dma_start(out=C[name], in_=t.ap())
            for p in ("lm", "cm", "lv"):
                for l in ("w1t", "w2t", "w3t"):
                    shape = [128, 1] if (p, l) == ("lv", "w3t") else [128, 128]
                    C[f"{p}_{l}"] = cp.tile(shape, dt.bfloat16, name=f"{p}_{l}")
                    nc.scalar.dma_start(out=C[f"{p}_{l}"], in_=w[f"{p}_{l}"].ap())
                for l in ("b1", "b2"):
                    C[f"{p}_{l}"] = cp.tile([128, 1], dt.float32, name=f"{p}_{l}")
                    nc.scalar.dma_start(out=C[f"{p}_{l}"], in_=w[f"{p}_{l}"].ap())
            for name, t, shape, dty in [
                ("cu_b", cu_b, [128, 4], dt.float32),
                ("lu_b", lu_b, [128, 4], dt.float32),
                ("lm_b3r", lm_b3r, [1, 512], dt.bfloat16),
                ("cm_b3r", cm_b3r, [1, 512], dt.bfloat16),
                ("ones1", ones1, [1, 128], dt.bfloat16),
                ("u0", u0, [1, 128], dt.bfloat16),
                ("sb1r", sb1r, [1, NCL], dt.bfloat16),
                ("colb", colb, [128, NCL], dt.bfloat16),
                ("rowb", rowb, [128, NL], dt.bfloat16),
                ("rowsc", rowsc, [128, 8], dt.float32),
                ("colsc", colsc, [128, 16], dt.float32),
            ]:
                C[name] = cp.tile(shape, dty, name=name)
                nc.scalar.dma_start(out=C[name], in_=t.ap())

            # ---- states ----
            Lh_pp = [sp.tile([128, NL], dt.bfloat16, name="Lh_a"),
                     sp.tile([128, NL], dt.bfloat16, name="Lh_b")]
            Ch = sp.tile([128, NCL], dt.bfloat16, name="Ch")
            Lc = sp.tile([128, NL], dt.bfloat16, name="Lc")
            Cc = sp.tile([128, NCL], dt.bfloat16, name="Cc")
            nc.scalar.dma_start(out=Lh_pp[0], in_=lh0.ap())
            nc.scalar.dma_start(out=Ch, in_=ch0.ap())
            nc.vector.memset(Lc, 0.0)
            nc.vector.memset(Cc, 0.0)

            # ---- resident slices of B: clause-chunk 0 of b1 (both phases)
            # plus a few pinned b2 slab-halves (cuts per-round streaming) ----
            b1res = [cp.tile([128, 32 * 512], dt.float8e4, name=f"b1res{h}")
                     for h in range(2)]
            for h in range(2):
                nc.sync.dma_start(out=b1res[h], in_=b1.ap()[0, h])
            PIN = [(0, 0, 0, 0), (0, 1, 0, 0), (1, 0, 0, 0), (1, 1, 0, 0), (0, 0, 1, 0)]
            b2res = {}
            for (pn, ph, psl, ps2) in PIN:
                tpin = cp.tile([128, 16 * 512], dt.float8e4,
                               name=f"b2res{pn}{ph}{psl}{ps2}")
                nc.sync.dma_start(
                    out=tpin,
                    in_=b2.ap()[pn, ph, psl][:, 16 * 512 * ps2:
                                             16 * 512 * (ps2 + 1)])
                b2res[(pn, ph, psl, ps2)] = tpin

            # ---- round-persistent work tiles ----
            lpre_img = wp.tile([128, NL], dt.float8e4, name="lpre_img")
            cpre_img = wp.tile([128, NCL], dt.float8e4, name="cpre_img")
            # gathered message halves: lpre_half[h] holds, for every core kk,
            # its chunk-h message tiles (4 tiles of 128 lits each) at columns
            # [512*kk, 512*kk+512); cpre_half[h] likewise with 8 tiles of 128
            # clauses at [1024*kk, 1024*kk+1024).
            lpre_half = [wp.tile([128, 8 * 512], dt.float8e4, name=f"lpre_h{h}")
                         for h in range(2)]
            cpre_half = [wp.tile([128, 8 * 1024], dt.float8e4, name=f"cpre_h{h}")
                         for h in range(2)]
            ag1_in = [dp.tile([128, 512], dt.float8e4, name=f"ag1{h}_in")
                      for h in range(2)]
            ag2_in = [dp.tile([128, 512], dt.float8e4, name=f"ag2{q}_in")
                      for q in range(4)]

            def mlp3_chunk(src, w1t, b1_, w2t, b2_, w3t, b3r, img, nn, sc):
                """3-layer MLP on one 512-col chunk; the 3rd layer transposes
                tile-wise into `img` and folds b3 in as a rank-1 matmul so the
                fp8 messages carry the full affine output (no gate-side
                correction needed)."""
                tag_sfx = "m"
                h1 = kp.tile([128, 512], dt.bfloat16, tag="mh1", bufs=2,
                             name=f"mh1_{nn}")
                h2 = kp.tile([128, 512], dt.bfloat16, tag="mh2", bufs=2,
                             name=f"mh2_{nn}")
                ps = psm.tile([128, 512], dt.float32, tag=tag_sfx,
                              name=f"mm1_{nn}")
                nc.tensor.matmul(ps, w1t, src, start=True, stop=True)
                nc.scalar.activation(h1, ps, AF.Relu, bias=b1_)
                ps = psm.tile([128, 512], dt.float32, tag=tag_sfx,
                              name=f"mm2_{nn}")
                nc.tensor.matmul(ps, w2t, h1, start=True, stop=True)
                nc.scalar.activation(h2, ps, AF.Relu, bias=b2_)
                ps3 = psm.tile([128, 512], dt.float32, tag=tag_sfx,
                               name=f"mm3_{nn}")
                for jj in range(4):
                    nc.tensor.matmul(ps3[:, 128 * jj:128 * (jj + 1)],
                                     h2[:, 128 * jj:128 * (jj + 1)], w3t,
                                     start=(jj == 0), stop=False,
                                     skip_group_check=True)
                nc.tensor.matmul(ps3, C["ones1"], b3r,
                                 start=False, stop=True, skip_group_check=True)
                for jj in range(4):
                    j = 4 * nn + jj
                    nc.scalar.activation(img[:, 128 * j:128 * (j + 1)],
                                         ps3[:, 128 * jj:128 * (jj + 1)],
                                         AF.Copy, scale=sc[:, j:j + 1])

            def l_msg_chunk(Lh_src, nn):
                """L-message for lit chunk nn (512 lits) + AG1-half kickoff."""
                sl = slice(512 * nn, 512 * (nn + 1))
                mlp3_chunk(Lh_src[:, sl], C["lm_w1t"], C["lm_b1"],
                           C["lm_w2t"], C["lm_b2"], C["lm_w3t"], C["lm_b3r"],
                           lpre_img, nn, C["rowsc"])
                nc.gpsimd.dma_start(out=ag1_in[nn], in_=lpre_img[:, sl])
                nc.gpsimd.collective_compute(
                    "AllGather", ALU.bypass, replica_groups=rg,
                    ins=[ag1_in[nn].opt()], outs=[ag1_out[nn].ap().opt()])

            def land_ag1(h):
                for kk in range(0, NCORES, 2):
                    nc.gpsimd.dma_start(
                        out=lpre_half[h][:, 512 * kk:512 * (kk + 2)]
                        .rearrange("p (k c) -> p k c", k=2),
                        in_=ag1_out[h].ap()[kk:kk + 2]
                        .rearrange("k p c -> p k c"))

            def c_msg_chunk(nn):
                """C-message for clause chunk nn (512 clauses)."""
                sl = slice(512 * nn, 512 * (nn + 1))
                mlp3_chunk(Ch[:, sl], C["cm_w1t"], C["cm_b1"],
                           C["cm_w2t"], C["cm_b2"], C["cm_w3t"], C["cm_b3r"],
                           cpre_img, nn, C["colsc"])
                nc.gpsimd.dma_start(out=ag2_in[nn], in_=cpre_img[:, sl])
                nc.gpsimd.collective_compute(
                    "AllGather", ALU.bypass, replica_groups=rg,
                    ins=[ag2_in[nn].opt()], outs=[ag2_out[nn].ap().opt()])

            def land_ag2(h):
                # phase h consumes clause chunks {2h, 2h+1}: quarter-gather q
                # lands at columns 1024*kk + 512*(q%2); two cores per DMA via
                # a strided 4D view (halves the serial gpsimd issue time)
                for q in (2 * h, 2 * h + 1):
                    v = cpre_half[h].rearrange("p (a b c) -> p a b c",
                                               a=NCORES, b=2)
                    for kk in range(0, NCORES, 2):
                        nc.gpsimd.dma_start(
                            out=v[:, kk:kk + 2, q % 2, :],
                            in_=ag2_out[q].ap()[kk:kk + 2]
                            .rearrange("k p c -> p k c"))

            def lstm_chunk(which, cc, xin, Lh_src=None, Lh_dst=None):
                """LSTM gate + state update for one 512-col chunk."""
                sl = slice(512 * cc, 512 * (cc + 1))
                gts = []
                for g in range(4):
                    gs = slice(128 * g, 128 * (g + 1))
                    ps = psg.tile([128, 512], dt.float32, tag=f"g{g % 2}",
                                  name=f"ps_{which}_{cc}_{g}")
                    if which == "c":
                        nc.tensor.matmul(ps, C["cu_wt"][:, gs], xin,
                                         start=True, stop=False,
                                         skip_group_check=True)
                        nc.tensor.matmul(ps, C["cu_ut"][:, gs], Ch[:, sl],
                                         start=False, stop=True,
                                         skip_group_check=True)
                        bias = C["cu_b"][:, g:g + 1]
                    else:
                        flip_sl = slice(512 * (1 - cc), 512 * (2 - cc))
                        nc.tensor.matmul(ps, C["lu_wcl"][:, gs], xin,
                                         start=True, stop=False,
                                         skip_group_check=True)
                        nc.tensor.matmul(ps, C["lu_wfl"][:, gs],
                                         Lh_src[:, flip_sl],
                                         start=False, stop=False,
                                         skip_group_check=True)
                        nc.tensor.matmul(ps, C["lu_ut"][:, gs], Lh_src[:, sl],
                                         start=False, stop=True,
                                         skip_group_check=True)
                        bias = C["lu_b"][:, g:g + 1]
                    gt = kp.tile([128, 512], dt.bfloat16, tag=f"gate{g}",
                                 bufs=1, name=f"gt_{which}_{cc}_{g}")
                    nc.scalar.activation(gt, ps,
                                         AF.Tanh if g == 2 else AF.Sigmoid,
                                         bias=bias)
                    gts.append(gt)
                cell = Cc if which == "c" else Lc
                hout = Ch if which == "c" else Lh_dst
                t1 = kp.tile([128, 512], dt.bfloat16, tag="t1", bufs=1,
                             name=f"t1_{which}_{cc}")
                t2 = kp.tile([128, 512], dt.bfloat16, tag="t2", bufs=1,
                             name=f"t2_{which}_{cc}")
                nc.vector.tensor_tensor(out=t1, in0=gts[1], in1=cell[:, sl],
                                        op=ALU.mult)
                nc.vector.tensor_tensor(out=t2, in0=gts[0], in1=gts[2],
                                        op=ALU.mult)
                nc.vector.tensor_tensor(out=cell[:, sl], in0=t1, in1=t2,
                                        op=ALU.add)
                t3 = kp.tile([128, 512], dt.bfloat16, tag="t3", bufs=1,
                             name=f"t3_{which}_{cc}")
                nc.scalar.activation(t3, cell[:, sl], AF.Tanh)
                nc.vector.tensor_tensor(out=hout[:, sl], in0=gts[3], in1=t3,
                                        op=ALU.mult)

            # Manual phase pinning: monotonically increasing scheduler-sim
            # timestamps force the emitted per-engine instruction order to
            # follow the hand-pipelined phase order. Without this, the
            # scheduler's naive collective cost model emits AG-completion
            # waits (landing copies) ahead of the next collective's trigger
            # on the gpsimd queue, head-blocking it for ~15us per round.
            _ph = [0]

            def phase():
                _ph[0] += 1
                return tc.tile_wait_until(_ph[0])

            # (no prologue: round 0's dir-1 collapses to rank-1 matmuls, so
            # no round-0 L messages or AG1s are needed)

            for r in range(rounds):
                Lh = Lh_pp[r % 2]
                Lh_new = Lh_pp[(r + 1) % 2]

                # ===== dir-1, group-major: clause chunks {0,1} are fully
                # contracted (h=0 then h=1) and their C side run first, so
                # AG2a kicks at ~50% of the C-phase with the whole second
                # group as its in-flight cover; group {2,3} then feeds AG2b,
                # which flies over dir-2 h=0 =====
                ps1 = [psd.tile([128, 512], dt.float32, tag=f"d{nn}",
                                name=f"ps1_{r}_{nn}") for nn in range(4)]

                def d1_contract(nn, h):
                    for s2 in range(2):
                        if nn == 0:
                            b1t = b1res[h][:, 16 * 512 * s2:
                                           16 * 512 * (s2 + 1)]
                        else:
                            b1t = bp.tile([128, 16 * 512], dt.float8e4,
                                          tag="b1", bufs=3,
                                          name=f"b1_{r}_{nn}_{h}_{s2}")
                            nc.sync.dma_start(
                                out=b1t,
                                in_=b1.ap()[nn, h][:, 16 * 512 * s2:
                                                   16 * 512 * (s2 + 1)])
                        for ttp in range(8):
                            tt = 16 * s2 + 2 * ttp
                            lhsT = lpre_half[h][:, 128 * tt:128 * (tt + 2)] \
                                .rearrange("p (e d) -> p e d", e=2)
                            rhs = b1t[:, 1024 * ttp:1024 * (ttp + 1)] \
                                .rearrange("p (e c) -> p e c", e=2)
                            nc.tensor.matmul(
                                ps1[nn], lhsT, rhs,
                                start=(h == 0 and tt == 0),
                                stop=(h == 1 and tt == 30),
                                perf_mode=mybir.MatmulPerfMode.DoubleRow,
                                skip_group_check=True)

                def c_xin(cn):
                    # hoisted ahead of the LSTM chains so the vector queue
                    # never head-blocks a gate matmul on a late xin
                    sl = slice(512 * cn, 512 * (cn + 1))
                    xin = kp.tile([128, 512], dt.bfloat16, tag="xin",
                                  bufs=2, name=f"lcs_{r}_{cn}")
                    nc.vector.tensor_tensor(out=xin, in0=ps1[cn],
                                            in1=C["colb"][:, sl],
                                            op=ALU.mult)
                    return xin

                def c_rest(cn, xin):
                    lstm_chunk("c", cn, xin)
                    c_msg_chunk(cn)

                for g in range(2):
                    n0, n1 = 2 * g, 2 * g + 1
                    if r == 0:
                        with phase():
                            for cn in (n0, n1):
                                sl = slice(512 * cn, 512 * (cn + 1))
                                nc.tensor.matmul(ps1[cn], C["u0"],
                                                 C["sb1r"][0:1, sl],
                                                 start=True, stop=True,
                                                 skip_group_check=True)
                            x0 = c_xin(n0)
                            x1 = c_xin(n1)
                            c_rest(n0, x0)
                            c_rest(n1, x1)      # kicks AG2a / AG2b
                    elif g == 0:
                        with phase():
                            land_ag1(0)
                        with phase():
                            d1_contract(n0, 0)
                            d1_contract(n1, 0)
                        with phase():
                            land_ag1(1)
                        with phase():
                            d1_contract(n0, 1)
                            x0 = c_xin(n0)
                            d1_contract(n1, 1)  # c_rest(n0) hides under this
                            x1 = c_xin(n1)
                            c_rest(n0, x0)
                            c_rest(n1, x1)      # kicks AG2a
                    else:
                        with phase():
                            d1_contract(n0, 0)
                            d1_contract(n1, 0)
                            d1_contract(n0, 1)
                            x0 = c_xin(n0)
                            d1_contract(n1, 1)
                            x1 = c_xin(n1)
                            c_rest(n0, x0)
                            c_rest(n1, x1)      # kicks AG2b

                # ===== dir-2, phase-major: h=0 contracts the AG2a tiles for
                # both lit chunks while AG2b flies; h=1 completes each chunk
                # and runs the L side, kicking the next round's AG1 halves =====
                ps2 = [psd.tile([128, 512], dt.float32, tag=f"d{nn}",
                                name=f"ps2_{r}_{nn}") for nn in range(2)]
                lxin = [None, None]
                for h in range(2):
                    with phase():
                        land_ag2(h)
                    with phase():
                        for nn in range(3 if h == 1 else 2):
                            if nn < 2:
                                for s in range(2):
                                    for s2 in range(2):
                                        if (nn, h, s, s2) in b2res:
                                            b2t = b2res[(nn, h, s, s2)]
                                        else:
                                            b2t = bp.tile([128, 16 * 512],
                                                          dt.float8e4,
                                                          tag="b2", bufs=4,
                                                          name=f"b2_{r}_{nn}_{h}_{s}_{s2}")
                                            nc.sync.dma_start(
                                                out=b2t,
                                                in_=b2.ap()[nn, h, s][:, 16 * 512 * s2:
                                                                      16 * 512 * (s2 + 1)])
                                        for ttp in range(8):
                                            tt = 16 * s2 + 2 * ttp
                                            lhsT = cpre_half[h][:, 4096 * s + 128 * tt:
                                                                4096 * s + 128 * (tt + 2)] \
                                                .rearrange("p (e d) -> p e d", e=2)
                                            rhs = b2t[:, 1024 * ttp:1024 * (ttp + 1)] \
                                                .rearrange("p (e c) -> p e c", e=2)
                                            nc.tensor.matmul(
                                                ps2[nn], lhsT, rhs,
                                                start=(h == 0 and s == 0 and tt == 0),
                                                stop=(h == 1 and s == 1 and tt == 30),
                                                perf_mode=mybir.MatmulPerfMode.DoubleRow,
                                                skip_group_check=True)
                            if h == 1 and nn < 2:
                                sl = slice(512 * nn, 512 * (nn + 1))
                                lxin[nn] = kp.tile([128, 512], dt.bfloat16,
                                                   tag="xin", bufs=2,
                                                   name=f"cls_{r}_{nn}")
                                nc.vector.tensor_tensor(out=lxin[nn],
                                                        in0=ps2[nn],
                                                        in1=C["rowb"][:, sl],
                                                        op=ALU.mult)
                            if h == 1 and nn > 0:
                                cn = nn - 1
                                lstm_chunk("l", cn, lxin[cn], Lh_src=Lh,
                                           Lh_dst=Lh_new)
                                if r < rounds - 1:
                                    l_msg_chunk(Lh_new, cn)

            # ===== vote MLP (bias of last layer added host-side) =====
            Lh_fin = Lh_pp[rounds % 2]
            vote_sb = wp.tile([1, NL], dt.float32, name="vote_sb")
            for nn in range(2):
                sl = slice(512 * nn, 512 * (nn + 1))
                vh1 = kp.tile([128, 512], dt.bfloat16, tag="mh1", bufs=2,
                              name=f"vh1_{nn}")
                vh2 = kp.tile([128, 512], dt.bfloat16, tag="mh2", bufs=2,
                              name=f"vh2_{nn}")
                ps = psm.tile([128, 512], dt.float32, tag="m", name=f"v1_{nn}")
                nc.tensor.matmul(ps, C["lv_w1t"], Lh_fin[:, sl],
                                 start=True, stop=True)
                nc.scalar.activation(vh1, ps, AF.Relu, bias=C["lv_b1"])
                ps = psm.tile([128, 512], dt.float32, tag="m", name=f"v2_{nn}")
                nc.tensor.matmul(ps, C["lv_w2t"], vh1,
                                 start=True, stop=True)
                nc.scalar.activation(vh2, ps, AF.Relu, bias=C["lv_b2"])
                ps = psm.tile([1, 512], dt.float32, tag="m", name=f"v3_{nn}")
                nc.tensor.matmul(ps, C["lv_w3t"], vh2,
                                 start=True, stop=True)
                nc.scalar.activation(vote_sb[0:1, sl], ps, AF.Copy)
            nc.scalar.dma_start(out=vote_out.ap(), in_=vote_sb)

    nc.compile()
    return nc


# ---------------------------------------------------------------------------
# host-side input preparation
# ---------------------------------------------------------------------------

def prep_inputs(inputs):
    g = {k: np.asarray(v) for k, v in inputs.items()}
    lit_idx = g["lit_idx"].astype(np.int64)
    clause_idx = g["clause_idx"].astype(np.int64)

    B = np.zeros((NL_TOT, NCL_TOT), np.bool_)
    B[lit_idx, clause_idx] = True
    degc = B.sum(0).astype(np.float32)
    degl = B.sum(1).astype(np.float32)
    col = (np.float32(1.0) / (np.sqrt(degc) + np.float32(1e-6))).astype(np.float32)
    row = (np.float32(1.0) / (np.sqrt(degl) + np.float32(1e-6))).astype(np.float32)
    # degree-0 rows/cols of A are structurally zero: clamp their scales so the
    # gained fp8 messages stay finite (mathematically identical result)
    col = np.where(degc > 0, col, np.float32(0)).astype(np.float32)
    row = np.where(degl > 0, row, np.float32(0)).astype(np.float32)

    # permuted lit order: core k <- [512k..512k+512) u [4096+512k..4096+512k+512)
    lit_order = np.concatenate(
        [np.concatenate([np.arange(512 * k, 512 * (k + 1)),
                         NV + np.arange(512 * k, 512 * (k + 1))])
         for k in range(NCORES)])
    Bu = B.astype(np.uint8) * FP8_ONE
    Bp = Bu[lit_order]                      # [8192, 16384] permuted rows
    row_p = row[lit_order]

    def b(x):
        return np.ascontiguousarray(np.asarray(x, np.float32)).astype(bf16)

    # round-0 shortcut constants: u0 = lm_mlp(lh0_vec)+lm_b3 (exact, host),
    # sb1 = GAIN * (row @ B) per clause
    lh0_vec = np.asarray(g["L_init_w"], np.float32)[:, 0] + np.asarray(
        g["L_init_b"], np.float32)
    _h = np.maximum(np.asarray(g["lm_w1"], np.float32) @ lh0_vec
                    + np.asarray(g["lm_b1"], np.float32), 0)
    _h = np.maximum(np.asarray(g["lm_w2"], np.float32) @ _h
                    + np.asarray(g["lm_b2"], np.float32), 0)
    u0_vec = np.asarray(g["lm_w3"], np.float32) @ _h + np.asarray(
        g["lm_b3"], np.float32)
    sb1_full = GAIN * (row @ B.astype(np.float32))

    common = {
        "lm_w1t": b(g["lm_w1"].T), "lm_w2t": b(g["lm_w2"].T), "lm_w3t": b(g["lm_w3"].T),
        "cm_w1t": b(g["cm_w1"].T), "cm_w2t": b(g["cm_w2"].T), "cm_w3t": b(g["cm_w3"].T),
        "lv_w1t": b(g["lv_w1"].T), "lv_w2t": b(g["lv_w2"].T), "lv_w3t": b(g["lv_w3"].T),
        "lm_b1": np.asarray(g["lm_b1"], np.float32).reshape(128, 1),
        "lm_b2": np.asarray(g["lm_b2"], np.float32).reshape(128, 1),
        "cm_b1": np.asarray(g["cm_b1"], np.float32).reshape(128, 1),
        "cm_b2": np.asarray(g["cm_b2"], np.float32).reshape(128, 1),
        "lv_b1": np.asarray(g["lv_b1"], np.float32).reshape(128, 1),
        "lv_b2": np.asarray(g["lv_b2"], np.float32).reshape(128, 1),
        "cu_wt": b(g["cu_wih"].T), "cu_ut": b(g["cu_whh"].T),
        "lu_wcl": b(g["lu_wih"][:, :D].T), "lu_wfl": b(g["lu_wih"][:, D:].T),
        "lu_ut": b(g["lu_whh"].T),
        "cu_b": np.asarray(g["cu_bih"] + g["cu_bhh"], np.float32).reshape(4, 128).T.copy(),
        "lu_b": np.asarray(g["lu_bih"] + g["lu_bhh"], np.float32).reshape(4, 128).T.copy(),
        "lm_b3r": b(np.tile(np.asarray(g["lm_b3"], np.float32), 4)).reshape(1, 512),
        "cm_b3r": b(np.tile(np.asarray(g["cm_b3"], np.float32), 4)).reshape(1, 512),
        "ones1": np.ones((1, 128), dtype=bf16),
        "u0": b(u0_vec).reshape(1, 128),
        "lh0": np.ascontiguousarray(np.broadcast_to(
            np.asarray(g["L_init_w"][:, 0] + g["L_init_b"], np.float32)[:, None],
            (128, NL))).astype(bf16),
        "ch0": np.ascontiguousarray(np.broadcast_to(
            np.asarray(g["C_init_w"][:, 0] + g["C_init_b"], np.float32)[:, None],
            (128, NCL))).astype(bf16),
    }

    in_maps = []
    for k in range(NCORES):
        lsl = slice(NL * k, NL * (k + 1))
        csl = slice(NCL * k, NCL * (k + 1))
        # b1: Bp[:, csl] is [t(64)*128p rows, nn(4)*512c cols]
        #     [kk(8), jh(2), jj(4), p, nn, c] -> [nn, jh, p, kk, jj, c]
        X = Bp[:, csl].reshape(8, 2, 4, 128, 4, 512)
        b1k = np.ascontiguousarray(X.transpose(4, 1, 3, 0, 2, 5)).reshape(
            4, 2, 128, 32 * 512).view(f8)
        # b2: Bp[lsl, :].T is [T(128)*128p rows, nn(2)*512l cols]
        #     [ks(2), kk(4), h(2), j2(8), p, nn, l] -> [nn, h, ks, p, kk, j2, l]
        Y = Bp[lsl, :].T.reshape(2, 4, 2, 8, 128, 2, 512)
        b2k = np.ascontiguousarray(Y.transpose(5, 2, 0, 4, 1, 3, 6)).reshape(
            2, 2, 2, 128, 32 * 512).view(f8)
        m = dict(common)
        m.update({
            "b1": b1k,
            "b2": b2k,
            "sb1r": sb1_full[csl].astype(bf16).reshape(1, NCL),
            "colb": np.ascontiguousarray(
                np.broadcast_to(col[csl][None, :] / GAIN, (128, NCL))).astype(bf16),
            "rowb": np.ascontiguousarray(
                np.broadcast_to(row_p[lsl][None, :] / GAIN, (128, NL))).astype(bf16),
            "rowsc": np.ascontiguousarray(
                GAIN * row_p[lsl].reshape(8, 128).T).astype(np.float32),
            "colsc": np.ascontiguousarray(
                GAIN * col[csl].reshape(16, 128).T).astype(np.float32),
        })
        in_maps.append(m)
    return in_maps


def selfcheck_layouts(in_maps, lit_idx, clause_idx):
    """Random probes: device-layout b1/b2 entries vs the raw B matrix."""
    B = np.zeros((NL_TOT, NCL_TOT), np.uint8)
    B[lit_idx, clause_idx] = FP8_ONE
    lit_order = np.concatenate(
        [np.concatenate([np.arange(512 * k, 512 * (k + 1)),
                         NV + np.arange(512 * k, 512 * (k + 1))])
         for k in range(NCORES)])
    Bp = B[lit_order]
    rng = np.random.default_rng(1)
    for k in (0, 3):
        b1k = in_maps[k]["b1"].view(np.uint8).reshape(4, 2, 128, 32, 512)
        for _ in range(50):
            nn, h, p, tt, c = (rng.integers(4), rng.integers(2), rng.integers(128),
                               rng.integers(32), rng.integers(512))
            t = 8 * (tt // 4) + 4 * h + tt % 4
            want = Bp[128 * t + p, 2048 * k + 512 * nn + c]
            assert b1k[nn, h, p, tt, c] == want, (k, nn, h, p, tt, c)
        b2k = in_maps[k]["b2"].view(np.uint8).reshape(2, 2, 2, 128, 32, 512)
        for _ in range(50):
            nn, h, s, p, tt, c = (rng.integers(2), rng.integers(2), rng.integers(2),
                                  rng.integers(128), rng.integers(32), rng.integers(512))
            T = 16 * (4 * s + tt // 8) + 8 * h + tt % 8
            want = Bp[1024 * k + 512 * nn + c, 128 * T + p]
            assert b2k[nn, h, s, p, tt, c] == want, (k, nn, h, s, p, tt, c)


_PROGRAM_CACHE = {}


def _get_program(rounds):
    if rounds not in _PROGRAM_CACHE:
        _PROGRAM_CACHE[rounds] = build_program(rounds)
    return _PROGRAM_CACHE[rounds]


def run_device(inputs, trace=False, rounds=None, **kw):
    if rounds is None:
        rounds = int(inputs.get("n_rounds", 16))
    in_maps = prep_inputs(inputs)
    nc = _get_program(rounds)
    res = bass_utils.run_bass_kernel_spmd(
        nc, in_maps, core_ids=list(range(NCORES)), trace=trace, **kw)
    return res


def assemble_votes(res_results, lv_b3):
    votes = np.stack([np.asarray(res_results[k]["vote"]).reshape(NL)
                      for k in range(NCORES)])   # [8, 1024]
    vote = votes + np.float32(lv_b3)
    pos = vote[:, :512].reshape(NV)              # var v -> core v//512
    neg = vote[:, 512:].reshape(NV)
    vj = np.stack([pos, neg], axis=1)            # [4096, 2]
    return vj.reshape(32, -1).mean(axis=1).astype(np.float32)


def kernel(**inputs) -> np.ndarray:
    res = run_device(inputs)
    return assemble_votes(res.results, np.asarray(inputs["lv_b3"]).reshape(-1)[0])



# revision 58
# speedup vs baseline: 1.0588x; 1.0155x over previous
"""NeuroSAT message-passing kernel for 8 Trainium2 NeuronCores (Bass/Tile).

Strategy
--------
The dense adjacency factors as A = D_row @ B @ D_col with B binary, so B
streams from HBM in fp8 (1.0/0.0 are exact in e4m3) as the *moving* matmul
operand against bf16 stationary message tiles; the degree scalings are
per-partition activation scales / free tensor_tensor multiplies at PSUM
eviction, and the (scaling-entangled) final-layer MLP biases become rank-1
K=1 matmul corrections accumulated straight into the LSTM gate PSUMs.

Sharding (8 cores):
  - clauses: core k owns [2048k, 2048k+2048)
  - literals: core k owns [512k, 512k+512) u [4096+512k, 4096+512k+512)
    (a positive block and its negation block, so NeuroSAT's "flip" is a
    local slice swap instead of a cross-core exchange)
All row-wise ops (MLPs, LSTMs) run on the local shard in feature-major
layout [dim(128) x rows]; the two A-applications per round contract over
the full lit/clause axes, fed by AllGathers of the scaled row-major
L/C messages. Each AllGather is split in two and the contraction loops
are ordered chunk-major so collectives and B-streaming DMAs hide under
the previous chunk's matmuls (keeps TensorE warm through the round).
"""
import sys

sys.path.insert(0, "/opt/trn_rl_repo")

import numpy as np
import ml_dtypes

import concourse.bass as bass
import concourse.mybir as mybir
import concourse.tile as tile
from concourse import bacc
from concourse import bass_utils

dt = mybir.dt
AF = mybir.ActivationFunctionType
ALU = mybir.AluOpType
bf16 = ml_dtypes.bfloat16
f8 = ml_dtypes.float8_e4m3

NCORES = 8
D = 128
NL_TOT, NCL_TOT, NV = 8192, 16384, 4096
NL = NL_TOT // NCORES      # 1024 lits per core
NCL = NCL_TOT // NCORES    # 2048 clauses per core
KT1 = NL_TOT // 128        # 64  k-tiles for A^T @ Lpre
KT2 = NCL_TOT // 128       # 128 k-tiles for A @ Cpre
FP8_ONE = 0x38             # bit pattern of 1.0 in float8_e4m3
GAIN = np.float32(128.0)   # power-of-2 pre-scale keeping fp8 messages normal-range


# ---------------------------------------------------------------------------
# device program
# ---------------------------------------------------------------------------

def build_program(rounds: int):
    nc = bacc.Bacc("TRN2", target_bir_lowering=False, debug=False,
                   num_devices=NCORES)

    def inp(name, shape, dty):
        return nc.dram_tensor(name, list(shape), dty, kind="ExternalInput")

    # B streams, pre-tiled to match the chunk-major contraction loops:
    # b1[nn, h]: slab of 32 k-tiles [128p, 32tt, 512c] covering clause chunk
    #            nn, lit-tile phase h (tt = 4k + jj, global tile t = 8k+4h+jj)
    # b2[nn, h, s]: slab of 32 k-tiles [128p, 32tt, 512l] covering lit chunk
    #            nn, clause-tile phase h, half s (tt=8kk+j2, T=16(4s+kk)+8h+j2)
    b1 = inp("b1", [4, 2, 128, 32 * 512], dt.float8e4)
    b2 = inp("b2", [2, 2, 2, 128, 32 * 512], dt.float8e4)
    w = {}
    for p in ("lm", "cm", "lv"):
        for l in ("w1t", "w2t", "w3t"):
            shape = [128, 1] if (p, l) == ("lv", "w3t") else [128, 128]
            w[f"{p}_{l}"] = inp(f"{p}_{l}", shape, dt.bfloat16)
        for l in ("b1", "b2"):
            w[f"{p}_{l}"] = inp(f"{p}_{l}", [128, 1], dt.float32)
    cu_wt = inp("cu_wt", [128, 512], dt.bfloat16)      # cu_wih.T
    cu_ut = inp("cu_ut", [128, 512], dt.bfloat16)      # cu_whh.T
    cu_b = inp("cu_b", [128, 4], dt.float32)
    lu_wcl = inp("lu_wcl", [128, 512], dt.bfloat16)    # lu_wih[:, :128].T
    lu_wfl = inp("lu_wfl", [128, 512], dt.bfloat16)    # lu_wih[:, 128:].T
    lu_ut = inp("lu_ut", [128, 512], dt.bfloat16)      # lu_whh.T
    lu_b = inp("lu_b", [128, 4], dt.float32)
    lm_b3r = inp("lm_b3r", [1, 512], dt.bfloat16)      # lm_b3 tiled 4x
    cm_b3r = inp("cm_b3r", [1, 512], dt.bfloat16)      # cm_b3 tiled 4x
    ones1 = inp("ones1", [1, 128], dt.bfloat16)
    # round-0 shortcut: Lh0 is a broadcast vector, so round 0's dir-1 output
    # is rank-1: ps1 = u0 (x) sb1r with u0 = mlp(lh0)+b3 (host, exact) and
    # sb1r = GAIN * (B^T row) over my clauses
    u0 = inp("u0", [1, 128], dt.bfloat16)
    sb1r = inp("sb1r", [1, NCL], dt.bfloat16)
    colb = inp("colb", [128, NCL], dt.bfloat16)         # col bcast over partitions
    rowb = inp("rowb", [128, NL], dt.bfloat16)          # row bcast over partitions
    rowsc = inp("rowsc", [128, 8], dt.float32)         # row, per lit-tile column
    colsc = inp("colsc", [128, 16], dt.float32)        # col, per clause-tile column
    lh0 = inp("lh0", [128, NL], dt.bfloat16)
    ch0 = inp("ch0", [128, NCL], dt.bfloat16)

    vote_out = nc.dram_tensor("vote", [1, NL], dt.float32, kind="ExternalOutput")
    # per-phase AllGather buffers: AG1a/b carry lit chunks 0/1 (the tiles
    # dir-1 phase h=0/1 contracts); AG2a/b carry clause chunks {0,1}/{2,3}
    # (what dir-2 phase h=0/1 contracts).
    ag1_out = [nc.dram_tensor(f"ag1{h}_out", [NCORES, 128, 512], dt.float8e4,
                              addr_space="Shared") for h in range(2)]
    ag2_out = [nc.dram_tensor(f"ag2{q}_out", [NCORES, 128, 512], dt.float8e4,
                              addr_space="Shared") for q in range(4)]
    rg = [list(range(NCORES))]

    with tile.TileContext(nc) as tc:
        with (
            tc.tile_pool(name="const", bufs=1) as cp,
            tc.tile_pool(name="state", bufs=1) as sp,
            tc.tile_pool(name="work", bufs=1) as wp,
            tc.tile_pool(name="chunk", bufs=2) as kp,
            tc.tile_pool(name="bstream", bufs=4) as bp,
            tc.tile_pool(name="psd", bufs=1, space="PSUM") as psd,
            tc.tile_pool(name="psg", bufs=1, space="PSUM") as psg,
            tc.tile_pool(name="psm", bufs=2, space="PSUM") as psm,
            tc.tile_pool(name="dram", bufs=1, space="DRAM") as dp,
        ):
            # ---- constants into SBUF ----
            C = {}
            for name, t in [
                ("cu_wt", cu_wt), ("cu_ut", cu_ut), ("lu_wcl", lu_wcl),
                ("lu_wfl", lu_wfl), ("lu_ut", lu_ut),
            ]:
                C[name] = cp.tile([128, 512], dt.bfloat16, name=name)
                nc.scalar.dma_start(out=C[name], in_=t.ap())
            for p in ("lm", "cm", "lv"):
                for l in ("w1t", "w2t", "w3t"):
                    shape = [128, 1] if (p, l) == ("lv", "w3t") else [128, 128]
                    C[f"{p}_{l}"] = cp.tile(shape, dt.bfloat16, name=f"{p}_{l}")
                    nc.scalar.dma_start(out=C[f"{p}_{l}"], in_=w[f"{p}_{l}"].ap())
                for l in ("b1", "b2"):
                    C[f"{p}_{l}"] = cp.tile([128, 1], dt.float32, name=f"{p}_{l}")
                    nc.scalar.dma_start(out=C[f"{p}_{l}"], in_=w[f"{p}_{l}"].ap())
            for name, t, shape, dty in [
                ("cu_b", cu_b, [128, 4], dt.float32),
                ("lu_b", lu_b, [128, 4], dt.float32),
                ("lm_b3r", lm_b3r, [1, 512], dt.bfloat16),
                ("cm_b3r", cm_b3r, [1, 512], dt.bfloat16),
                ("ones1", ones1, [1, 128], dt.bfloat16),
                ("u0", u0, [1, 128], dt.bfloat16),
                ("sb1r", sb1r, [1, NCL], dt.bfloat16),
                ("colb", colb, [128, NCL], dt.bfloat16),
                ("rowb", rowb, [128, NL], dt.bfloat16),
                ("rowsc", rowsc, [128, 8], dt.float32),
                ("colsc", colsc, [128, 16], dt.float32),
            ]:
                C[name] = cp.tile(shape, dty, name=name)
                nc.scalar.dma_start(out=C[name], in_=t.ap())

            # ---- states ----
            Lh_pp = [sp.tile([128, NL], dt.bfloat16, name="Lh_a"),
                     sp.tile([128, NL], dt.bfloat16, name="Lh_b")]
            Ch = sp.tile([128, NCL], dt.bfloat16, name="Ch")
            Lc = sp.tile([128, NL], dt.bfloat16, name="Lc")
            Cc = sp.tile([128, NCL], dt.bfloat16, name="Cc")
            nc.scalar.dma_start(out=Lh_pp[0], in_=lh0.ap())
            nc.scalar.dma_start(out=Ch, in_=ch0.ap())
            nc.vector.memset(Lc, 0.0)
            nc.vector.memset(Cc, 0.0)

            # ---- resident slices of B: clause-chunk 0 of b1 (both phases)
            # plus a few pinned b2 slab-halves (cuts per-round streaming) ----
            b1res = [cp.tile([128, 32 * 512], dt.float8e4, name=f"b1res{h}")
                     for h in range(2)]
            for h in range(2):
                nc.sync.dma_start(out=b1res[h], in_=b1.ap()[0, h])
            PIN = [(0, 0, 0, 0), (0, 1, 0, 0), (1, 0, 0, 0), (1, 1, 0, 0), (0, 0, 1, 0)]
            b2res = {}
            for (pn, ph, psl, ps2) in PIN:
                tpin = cp.tile([128, 16 * 512], dt.float8e4,
                               name=f"b2res{pn}{ph}{psl}{ps2}")
                nc.sync.dma_start(
                    out=tpin,
                    in_=b2.ap()[pn, ph, psl][:, 16 * 512 * ps2:
                                             16 * 512 * (ps2 + 1)])
                b2res[(pn, ph, psl, ps2)] = tpin

            # ---- round-persistent work tiles ----
            lpre_img = wp.tile([128, NL], dt.float8e4, name="lpre_img")
            cpre_img = wp.tile([128, NCL], dt.float8e4, name="cpre_img")
            # gathered message halves: lpre_half[h] holds, for every core kk,
            # its chunk-h message tiles (4 tiles of 128 lits each) at columns
            # [512*kk, 512*kk+512); cpre_half[h] likewise with 8 tiles of 128
            # clauses at [1024*kk, 1024*kk+1024).
            lpre_half = [wp.tile([128, 8 * 512], dt.float8e4, name=f"lpre_h{h}")
                         for h in range(2)]
            cpre_half = [wp.tile([128, 8 * 1024], dt.float8e4, name=f"cpre_h{h}")
                         for h in range(2)]
            ag1_in = [dp.tile([128, 512], dt.float8e4, name=f"ag1{h}_in")
                      for h in range(2)]
            ag2_in = [dp.tile([128, 512], dt.float8e4, name=f"ag2{q}_in")
                      for q in range(4)]

            def mlp3_chunk(src, w1t, b1_, w2t, b2_, w3t, b3r, img, nn, sc):
                """3-layer MLP on one 512-col chunk; the 3rd layer transposes
                tile-wise into `img` and folds b3 in as a rank-1 matmul so the
                fp8 messages carry the full affine output (no gate-side
                correction needed)."""
                tag_sfx = "m"
                h1 = kp.tile([128, 512], dt.bfloat16, tag="mh1", bufs=2,
                             name=f"mh1_{nn}")
                h2 = kp.tile([128, 512], dt.bfloat16, tag="mh2", bufs=2,
                             name=f"mh2_{nn}")
                ps = psm.tile([128, 512], dt.float32, tag=tag_sfx,
                              name=f"mm1_{nn}")
                nc.tensor.matmul(ps, w1t, src, start=True, stop=True)
                nc.scalar.activation(h1, ps, AF.Relu, bias=b1_)
                ps = psm.tile([128, 512], dt.float32, tag=tag_sfx,
                              name=f"mm2_{nn}")
                nc.tensor.matmul(ps, w2t, h1, start=True, stop=True)
                nc.scalar.activation(h2, ps, AF.Relu, bias=b2_)
                ps3 = psm.tile([128, 512], dt.float32, tag=tag_sfx,
                               name=f"mm3_{nn}")
                for jj in range(4):
                    nc.tensor.matmul(ps3[:, 128 * jj:128 * (jj + 1)],
                                     h2[:, 128 * jj:128 * (jj + 1)], w3t,
                                     start=(jj == 0), stop=False,
                                     skip_group_check=True)
                nc.tensor.matmul(ps3, C["ones1"], b3r,
                                 start=False, stop=True, skip_group_check=True)
                for jj in range(4):
                    j = 4 * nn + jj
                    nc.scalar.activation(img[:, 128 * j:128 * (j + 1)],
                                         ps3[:, 128 * jj:128 * (jj + 1)],
                                         AF.Copy, scale=sc[:, j:j + 1])

            def l_msg_chunk(Lh_src, nn):
                """L-message for lit chunk nn (512 lits) + AG1-half kickoff."""
                sl = slice(512 * nn, 512 * (nn + 1))
                mlp3_chunk(Lh_src[:, sl], C["lm_w1t"], C["lm_b1"],
                           C["lm_w2t"], C["lm_b2"], C["lm_w3t"], C["lm_b3r"],
                           lpre_img, nn, C["rowsc"])
                nc.gpsimd.dma_start(out=ag1_in[nn], in_=lpre_img[:, sl])
                nc.gpsimd.collective_compute(
                    "AllGather", ALU.bypass, replica_groups=rg,
                    ins=[ag1_in[nn].opt()], outs=[ag1_out[nn].ap().opt()])

            def land_ag1(h):
                for kk in range(0, NCORES, 2):
                    nc.gpsimd.dma_start(
                        out=lpre_half[h][:, 512 * kk:512 * (kk + 2)]
                        .rearrange("p (k c) -> p k c", k=2),
                        in_=ag1_out[h].ap()[kk:kk + 2]
                        .rearrange("k p c -> p k c"))

            def c_msg_chunk(nn):
                """C-message for clause chunk nn (512 clauses)."""
                sl = slice(512 * nn, 512 * (nn + 1))
                mlp3_chunk(Ch[:, sl], C["cm_w1t"], C["cm_b1"],
                           C["cm_w2t"], C["cm_b2"], C["cm_w3t"], C["cm_b3r"],
                           cpre_img, nn, C["colsc"])
                nc.gpsimd.dma_start(out=ag2_in[nn], in_=cpre_img[:, sl])
                nc.gpsimd.collective_compute(
                    "AllGather", ALU.bypass, replica_groups=rg,
                    ins=[ag2_in[nn].opt()], outs=[ag2_out[nn].ap().opt()])

            def land_ag2(h):
                # phase h consumes clause chunks {2h, 2h+1}: quarter-gather q
                # lands at columns 1024*kk + 512*(q%2); two cores per DMA via
                # a strided 4D view (halves the serial gpsimd issue time)
                for q in (2 * h, 2 * h + 1):
                    v = cpre_half[h].rearrange("p (a b c) -> p a b c",
                                               a=NCORES, b=2)
                    for kk in range(0, NCORES, 2):
                        nc.gpsimd.dma_start(
                            out=v[:, kk:kk + 2, q % 2, :],
                            in_=ag2_out[q].ap()[kk:kk + 2]
                            .rearrange("k p c -> p k c"))

            def lstm_chunk(which, cc, xin, Lh_src=None, Lh_dst=None):
                """LSTM gate + state update for one 512-col chunk."""
                sl = slice(512 * cc, 512 * (cc + 1))
                gts = []
                for g in range(4):
                    gs = slice(128 * g, 128 * (g + 1))
                    ps = psg.tile([128, 512], dt.float32, tag=f"g{g % 2}",
                                  name=f"ps_{which}_{cc}_{g}")
                    if which == "c":
                        nc.tensor.matmul(ps, C["cu_wt"][:, gs], xin,
                                         start=True, stop=False,
                                         skip_group_check=True)
                        nc.tensor.matmul(ps, C["cu_ut"][:, gs], Ch[:, sl],
                                         start=False, stop=True,
                                         skip_group_check=True)
                        bias = C["cu_b"][:, g:g + 1]
                    else:
                        flip_sl = slice(512 * (1 - cc), 512 * (2 - cc))
                        nc.tensor.matmul(ps, C["lu_wcl"][:, gs], xin,
                                         start=True, stop=False,
                                         skip_group_check=True)
                        nc.tensor.matmul(ps, C["lu_wfl"][:, gs],
                                         Lh_src[:, flip_sl],
                                         start=False, stop=False,
                                         skip_group_check=True)
                        nc.tensor.matmul(ps, C["lu_ut"][:, gs], Lh_src[:, sl],
                                         start=False, stop=True,
                                         skip_group_check=True)
                        bias = C["lu_b"][:, g:g + 1]
                    gt = kp.tile([128, 512], dt.bfloat16, tag=f"gate{g}",
                                 bufs=1, name=f"gt_{which}_{cc}_{g}")
                    nc.scalar.activation(gt, ps,
                                         AF.Tanh if g == 2 else AF.Sigmoid,
                                         bias=bias)
                    gts.append(gt)
                cell = Cc if which == "c" else Lc
                hout = Ch if which == "c" else Lh_dst
                t1 = kp.tile([128, 512], dt.bfloat16, tag="t1", bufs=1,
                             name=f"t1_{which}_{cc}")
                t2 = kp.tile([128, 512], dt.bfloat16, tag="t2", bufs=1,
                             name=f"t2_{which}_{cc}")
                nc.vector.tensor_tensor(out=t1, in0=gts[1], in1=cell[:, sl],
                                        op=ALU.mult)
                nc.vector.tensor_tensor(out=t2, in0=gts[0], in1=gts[2],
                                        op=ALU.mult)
                nc.vector.tensor_tensor(out=cell[:, sl], in0=t1, in1=t2,
                                        op=ALU.add)
                t3 = kp.tile([128, 512], dt.bfloat16, tag="t3", bufs=1,
                             name=f"t3_{which}_{cc}")
                nc.scalar.activation(t3, cell[:, sl], AF.Tanh)
                nc.vector.tensor_tensor(out=hout[:, sl], in0=gts[3], in1=t3,
                                        op=ALU.mult)

            # Manual phase pinning: monotonically increasing scheduler-sim
            # timestamps force the emitted per-engine instruction order to
            # follow the hand-pipelined phase order. Without this, the
            # scheduler's naive collective cost model emits AG-completion
            # waits (landing copies) ahead of the next collective's trigger
            # on the gpsimd queue, head-blocking it for ~15us per round.
            _ph = [0]

            def phase():
                _ph[0] += 1
                return tc.tile_wait_until(_ph[0])

            # (no prologue: round 0's dir-1 collapses to rank-1 matmuls, so
            # no round-0 L messages or AG1s are needed)

            for r in range(rounds):
                Lh = Lh_pp[r % 2]
                Lh_new = Lh_pp[(r + 1) % 2]

                # ===== dir-1, group-major: clause chunks {0,1} are fully
                # contracted (h=0 then h=1) and their C side run first, so
                # AG2a kicks at ~50% of the C-phase with the whole second
                # group as its in-flight cover; group {2,3} then feeds AG2b,
                # which flies over dir-2 h=0 =====
                ps1 = [psd.tile([128, 512], dt.float32, tag=f"d{nn}",
                                name=f"ps1_{r}_{nn}") for nn in range(4)]

                def d1_contract(nn, h):
                    for s2 in range(2):
                        if nn == 0:
                            b1t = b1res[h][:, 16 * 512 * s2:
                                           16 * 512 * (s2 + 1)]
                        else:
                            b1t = bp.tile([128, 16 * 512], dt.float8e4,
                                          tag="b1", bufs=3,
                                          name=f"b1_{r}_{nn}_{h}_{s2}")
                            nc.sync.dma_start(
                                out=b1t,
                                in_=b1.ap()[nn, h][:, 16 * 512 * s2:
                                                   16 * 512 * (s2 + 1)])
                        for ttp in range(8):
                            tt = 16 * s2 + 2 * ttp
                            lhsT = lpre_half[h][:, 128 * tt:128 * (tt + 2)] \
                                .rearrange("p (e d) -> p e d", e=2)
                            rhs = b1t[:, 1024 * ttp:1024 * (ttp + 1)] \
                                .rearrange("p (e c) -> p e c", e=2)
                            nc.tensor.matmul(
                                ps1[nn], lhsT, rhs,
                                start=(h == 0 and tt == 0),
                                stop=(h == 1 and tt == 30),
                                perf_mode=mybir.MatmulPerfMode.DoubleRow,
                                skip_group_check=True)

                def c_xin(cn):
                    # hoisted ahead of the LSTM chains so the vector queue
                    # never head-blocks a gate matmul on a late xin
                    sl = slice(512 * cn, 512 * (cn + 1))
                    xin = kp.tile([128, 512], dt.bfloat16, tag="xin",
                                  bufs=2, name=f"lcs_{r}_{cn}")
                    nc.vector.tensor_tensor(out=xin, in0=ps1[cn],
                                            in1=C["colb"][:, sl],
                                            op=ALU.mult)
                    return xin

                def c_rest(cn, xin):
                    lstm_chunk("c", cn, xin)
                    c_msg_chunk(cn)

                for g in range(2):
                    n0, n1 = 2 * g, 2 * g + 1
                    if r == 0:
                        with phase():
                            for cn in (n0, n1):
                                sl = slice(512 * cn, 512 * (cn + 1))
                                nc.tensor.matmul(ps1[cn], C["u0"],
                                                 C["sb1r"][0:1, sl],
                                                 start=True, stop=True,
                                                 skip_group_check=True)
                            x0 = c_xin(n0)
                            x1 = c_xin(n1)
                            c_rest(n0, x0)
                            c_rest(n1, x1)      # kicks AG2a / AG2b
                    elif g == 0:
                        with phase():
                            land_ag1(0)
                        with phase():
                            d1_contract(n0, 0)
                            d1_contract(n1, 0)
                        with phase():
                            land_ag1(1)
                        with phase():
                            d1_contract(n0, 1)
                            x0 = c_xin(n0)
                            d1_contract(n1, 1)  # c_rest(n0) hides under this
                            x1 = c_xin(n1)
                            c_rest(n0, x0)
                            c_rest(n1, x1)      # kicks AG2a
                    else:
                        with phase():
                            d1_contract(n0, 0)
                            d1_contract(n0, 1)
                            x0 = c_xin(n0)
                            d1_contract(n1, 0)
                            c_rest(n0, x0)      # kicks AG2-q2 early
                            d1_contract(n1, 1)
                            x1 = c_xin(n1)
                            c_rest(n1, x1)      # kicks AG2-q3

                # ===== dir-2, phase-major: h=0 contracts the AG2a tiles for
                # both lit chunks while AG2b flies; h=1 completes each chunk
                # and runs the L side, kicking the next round's AG1 halves =====
                ps2 = [psd.tile([128, 512], dt.float32, tag=f"d{nn}",
                                name=f"ps2_{r}_{nn}") for nn in range(2)]
                lxin = [None, None]
                for h in range(2):
                    with phase():
                        land_ag2(h)
                    with phase():
                        for nn in range(3 if h == 1 else 2):
                            if nn < 2:
                                for s in range(2):
                                    for s2 in range(2):
                                        if (nn, h, s, s2) in b2res:
                                            b2t = b2res[(nn, h, s, s2)]
                                        else:
                                            b2t = bp.tile([128, 16 * 512],
                                                          dt.float8e4,
                                                          tag="b2", bufs=4,
                                                          name=f"b2_{r}_{nn}_{h}_{s}_{s2}")
                                            nc.sync.dma_start(
                                                out=b2t,
                                                in_=b2.ap()[nn, h, s][:, 16 * 512 * s2:
                                                                      16 * 512 * (s2 + 1)])
                                        for ttp in range(8):
                                            tt = 16 * s2 + 2 * ttp
                                            lhsT = cpre_half[h][:, 4096 * s + 128 * tt:
                                                                4096 * s + 128 * (tt + 2)] \
                                                .rearrange("p (e d) -> p e d", e=2)
                                            rhs = b2t[:, 1024 * ttp:1024 * (ttp + 1)] \
                                                .rearrange("p (e c) -> p e c", e=2)
                                            nc.tensor.matmul(
                                                ps2[nn], lhsT, rhs,
                                                start=(h == 0 and s == 0 and tt == 0),
                                                stop=(h == 1 and s == 1 and tt == 30),
                                                perf_mode=mybir.MatmulPerfMode.DoubleRow,
                                                skip_group_check=True)
                            if h == 1 and nn < 2:
                                sl = slice(512 * nn, 512 * (nn + 1))
                                lxin[nn] = kp.tile([128, 512], dt.bfloat16,
                                                   tag="xin", bufs=2,
                                                   name=f"cls_{r}_{nn}")
                                nc.vector.tensor_tensor(out=lxin[nn],
                                                        in0=ps2[nn],
                                                        in1=C["rowb"][:, sl],
                                                        op=ALU.mult)
                            if h == 1 and nn > 0:
                                cn = nn - 1
                                lstm_chunk("l", cn, lxin[cn], Lh_src=Lh,
                                           Lh_dst=Lh_new)
                                if r < rounds - 1:
                                    l_msg_chunk(Lh_new, cn)

            # ===== vote MLP (bias of last layer added host-side) =====
            Lh_fin = Lh_pp[rounds % 2]
            vote_sb = wp.tile([1, NL], dt.float32, name="vote_sb")
            for nn in range(2):
                sl = slice(512 * nn, 512 * (nn + 1))
                vh1 = kp.tile([128, 512], dt.bfloat16, tag="mh1", bufs=2,
                              name=f"vh1_{nn}")
                vh2 = kp.tile([128, 512], dt.bfloat16, tag="mh2", bufs=2,
                              name=f"vh2_{nn}")
                ps = psm.tile([128, 512], dt.float32, tag="m", name=f"v1_{nn}")
                nc.tensor.matmul(ps, C["lv_w1t"], Lh_fin[:, sl],
                                 start=True, stop=True)
                nc.scalar.activation(vh1, ps, AF.Relu, bias=C["lv_b1"])
                ps = psm.tile([128, 512], dt.float32, tag="m", name=f"v2_{nn}")
                nc.tensor.matmul(ps, C["lv_w2t"], vh1,
                                 start=True, stop=True)
                nc.scalar.activation(vh2, ps, AF.Relu, bias=C["lv_b2"])
                ps = psm.tile([1, 512], dt.float32, tag="m", name=f"v3_{nn}")
                nc.tensor.matmul(ps, C["lv_w3t"], vh2,
                                 start=True, stop=True)
                nc.scalar.activation(vote_sb[0:1, sl], ps, AF.Copy)
            nc.scalar.dma_start(out=vote_out.ap(), in_=vote_sb)

    nc.compile()
    return nc


# ---------------------------------------------------------------------------
# host-side input preparation
# ---------------------------------------------------------------------------

def prep_inputs(inputs):
    g = {k: np.asarray(v) for k, v in inputs.items()}
    lit_idx = g["lit_idx"].astype(np.int64)
    clause_idx = g["clause_idx"].astype(np.int64)

    B = np.zeros((NL_TOT, NCL_TOT), np.bool_)
    B[lit_idx, clause_idx] = True
    degc = B.sum(0).astype(np.float32)
    degl = B.sum(1).astype(np.float32)
    col = (np.float32(1.0) / (np.sqrt(degc) + np.float32(1e-6))).astype(np.float32)
    row = (np.float32(1.0) / (np.sqrt(degl) + np.float32(1e-6))).astype(np.float32)
    # degree-0 rows/cols of A are structurally zero: clamp their scales so the
    # gained fp8 messages stay finite (mathematically identical result)
    col = np.where(degc > 0, col, np.float32(0)).astype(np.float32)
    row = np.where(degl > 0, row, np.float32(0)).astype(np.float32)

    # permuted lit order: core k <- [512k..512k+512) u [4096+512k..4096+512k+512)
    lit_order = np.concatenate(
        [np.concatenate([np.arange(512 * k, 512 * (k + 1)),
                         NV + np.arange(512 * k, 512 * (k + 1))])
         for k in range(NCORES)])
    Bu = B.astype(np.uint8) * FP8_ONE
    Bp = Bu[lit_order]                      # [8192, 16384] permuted rows
    row_p = row[lit_order]

    def b(x):
        return np.ascontiguousarray(np.asarray(x, np.float32)).astype(bf16)

    # round-0 shortcut constants: u0 = lm_mlp(lh0_vec)+lm_b3 (exact, host),
    # sb1 = GAIN * (row @ B) per clause
    lh0_vec = np.asarray(g["L_init_w"], np.float32)[:, 0] + np.asarray(
        g["L_init_b"], np.float32)
    _h = np.maximum(np.asarray(g["lm_w1"], np.float32) @ lh0_vec
                    + np.asarray(g["lm_b1"], np.float32), 0)
    _h = np.maximum(np.asarray(g["lm_w2"], np.float32) @ _h
                    + np.asarray(g["lm_b2"], np.float32), 0)
    u0_vec = np.asarray(g["lm_w3"], np.float32) @ _h + np.asarray(
        g["lm_b3"], np.float32)
    sb1_full = GAIN * (row @ B.astype(np.float32))

    common = {
        "lm_w1t": b(g["lm_w1"].T), "lm_w2t": b(g["lm_w2"].T), "lm_w3t": b(g["lm_w3"].T),
        "cm_w1t": b(g["cm_w1"].T), "cm_w2t": b(g["cm_w2"].T), "cm_w3t": b(g["cm_w3"].T),
        "lv_w1t": b(g["lv_w1"].T), "lv_w2t": b(g["lv_w2"].T), "lv_w3t": b(g["lv_w3"].T),
        "lm_b1": np.asarray(g["lm_b1"], np.float32).reshape(128, 1),
        "lm_b2": np.asarray(g["lm_b2"], np.float32).reshape(128, 1),
        "cm_b1": np.asarray(g["cm_b1"], np.float32).reshape(128, 1),
        "cm_b2": np.asarray(g["cm_b2"], np.float32).reshape(128, 1),
        "lv_b1": np.asarray(g["lv_b1"], np.float32).reshape(128, 1),
        "lv_b2": np.asarray(g["lv_b2"], np.float32).reshape(128, 1),
        "cu_wt": b(g["cu_wih"].T), "cu_ut": b(g["cu_whh"].T),
        "lu_wcl": b(g["lu_wih"][:, :D].T), "lu_wfl": b(g["lu_wih"][:, D:].T),
        "lu_ut": b(g["lu_whh"].T),
        "cu_b": np.asarray(g["cu_bih"] + g["cu_bhh"], np.float32).reshape(4, 128).T.copy(),
        "lu_b": np.asarray(g["lu_bih"] + g["lu_bhh"], np.float32).reshape(4, 128).T.copy(),
        "lm_b3r": b(np.tile(np.asarray(g["lm_b3"], np.float32), 4)).reshape(1, 512),
        "cm_b3r": b(np.tile(np.asarray(g["cm_b3"], np.float32), 4)).reshape(1, 512),
        "ones1": np.ones((1, 128), dtype=bf16),
        "u0": b(u0_vec).reshape(1, 128),
        "lh0": np.ascontiguousarray(np.broadcast_to(
            np.asarray(g["L_init_w"][:, 0] + g["L_init_b"], np.float32)[:, None],
            (128, NL))).astype(bf16),
        "ch0": np.ascontiguousarray(np.broadcast_to(
            np.asarray(g["C_init_w"][:, 0] + g["C_init_b"], np.float32)[:, None],
            (128, NCL))).astype(bf16),
    }

    in_maps = []
    for k in range(NCORES):
        lsl = slice(NL * k, NL * (k + 1))
        csl = slice(NCL * k, NCL * (k + 1))
        # b1: Bp[:, csl] is [t(64)*128p rows, nn(4)*512c cols]
        #     [kk(8), jh(2), jj(4), p, nn, c] -> [nn, jh, p, kk, jj, c]
        X = Bp[:, csl].reshape(8, 2, 4, 128, 4, 512)
        b1k = np.ascontiguousarray(X.transpose(4, 1, 3, 0, 2, 5)).reshape(
            4, 2, 128, 32 * 512).view(f8)
        # b2: Bp[lsl, :].T is [T(128)*128p rows, nn(2)*512l cols]
        #     [ks(2), kk(4), h(2), j2(8), p, nn, l] -> [nn, h, ks, p, kk, j2, l]
        Y = Bp[lsl, :].T.reshape(2, 4, 2, 8, 128, 2, 512)
        b2k = np.ascontiguousarray(Y.transpose(5, 2, 0, 4, 1, 3, 6)).reshape(
            2, 2, 2, 128, 32 * 512).view(f8)
        m = dict(common)
        m.update({
            "b1": b1k,
            "b2": b2k,
            "sb1r": sb1_full[csl].astype(bf16).reshape(1, NCL),
            "colb": np.ascontiguousarray(
                np.broadcast_to(col[csl][None, :] / GAIN, (128, NCL))).astype(bf16),
            "rowb": np.ascontiguousarray(
                np.broadcast_to(row_p[lsl][None, :] / GAIN, (128, NL))).astype(bf16),
            "rowsc": np.ascontiguousarray(
                GAIN * row_p[lsl].reshape(8, 128).T).astype(np.float32),
            "colsc": np.ascontiguousarray(
                GAIN * col[csl].reshape(16, 128).T).astype(np.float32),
        })
        in_maps.append(m)
    return in_maps


def selfcheck_layouts(in_maps, lit_idx, clause_idx):
    """Random probes: device-layout b1/b2 entries vs the raw B matrix."""
    B = np.zeros((NL_TOT, NCL_TOT), np.uint8)
    B[lit_idx, clause_idx] = FP8_ONE
    lit_order = np.concatenate(
        [np.concatenate([np.arange(512 * k, 512 * (k + 1)),
                         NV + np.arange(512 * k, 512 * (k + 1))])
         for k in range(NCORES)])
    Bp = B[lit_order]
    rng = np.random.default_rng(1)
    for k in (0, 3):
        b1k = in_maps[k]["b1"].view(np.uint8).reshape(4, 2, 128, 32, 512)
        for _ in range(50):
            nn, h, p, tt, c = (rng.integers(4), rng.integers(2), rng.integers(128),
                               rng.integers(32), rng.integers(512))
            t = 8 * (tt // 4) + 4 * h + tt % 4
            want = Bp[128 * t + p, 2048 * k + 512 * nn + c]
            assert b1k[nn, h, p, tt, c] == want, (k, nn, h, p, tt, c)
        b2k = in_maps[k]["b2"].view(np.uint8).reshape(2, 2, 2, 128, 32, 512)
        for _ in range(50):
            nn, h, s, p, tt, c = (rng.integers(2), rng.integers(2), rng.integers(2),
                                  rng.integers(128), rng.integers(32), rng.integers(512))
            T = 16 * (4 * s + tt // 8) + 8 * h + tt % 8
            want = Bp[1024 * k + 512 * nn + c, 128 * T + p]
            assert b2k[nn, h, s, p, tt, c] == want, (k, nn, h, s, p, tt, c)


_PROGRAM_CACHE = {}


def _get_program(rounds):
    if rounds not in _PROGRAM_CACHE:
        _PROGRAM_CACHE[rounds] = build_program(rounds)
    return _PROGRAM_CACHE[rounds]


def run_device(inputs, trace=False, rounds=None, **kw):
    if rounds is None:
        rounds = int(inputs.get("n_rounds", 16))
    in_maps = prep_inputs(inputs)
    nc = _get_program(rounds)
    res = bass_utils.run_bass_kernel_spmd(
        nc, in_maps, core_ids=list(range(NCORES)), trace=trace, **kw)
    return res


def assemble_votes(res_results, lv_b3):
    votes = np.stack([np.asarray(res_results[k]["vote"]).reshape(NL)
                      for k in range(NCORES)])   # [8, 1024]
    vote = votes + np.float32(lv_b3)
    pos = vote[:, :512].reshape(NV)              # var v -> core v//512
    neg = vote[:, 512:].reshape(NV)
    vj = np.stack([pos, neg], axis=1)            # [4096, 2]
    return vj.reshape(32, -1).mean(axis=1).astype(np.float32)


def kernel(**inputs) -> np.ndarray:
    res = run_device(inputs)
    return assemble_votes(res.results, np.asarray(inputs["lv_b3"]).reshape(-1)[0])



# revision 59
# speedup vs baseline: 1.0602x; 1.0013x over previous
"""NeuroSAT message-passing kernel for 8 Trainium2 NeuronCores (Bass/Tile).

Strategy
--------
The dense adjacency factors as A = D_row @ B @ D_col with B binary, so B
streams from HBM in fp8 (1.0/0.0 are exact in e4m3) as the *moving* matmul
operand against bf16 stationary message tiles; the degree scalings are
per-partition activation scales / free tensor_tensor multiplies at PSUM
eviction, and the (scaling-entangled) final-layer MLP biases become rank-1
K=1 matmul corrections accumulated straight into the LSTM gate PSUMs.

Sharding (8 cores):
  - clauses: core k owns [2048k, 2048k+2048)
  - literals: core k owns [512k, 512k+512) u [4096+512k, 4096+512k+512)
    (a positive block and its negation block, so NeuroSAT's "flip" is a
    local slice swap instead of a cross-core exchange)
All row-wise ops (MLPs, LSTMs) run on the local shard in feature-major
layout [dim(128) x rows]; the two A-applications per round contract over
the full lit/clause axes, fed by AllGathers of the scaled row-major
L/C messages. Each AllGather is split in two and the contraction loops
are ordered chunk-major so collectives and B-streaming DMAs hide under
the previous chunk's matmuls (keeps TensorE warm through the round).
"""
import sys

sys.path.insert(0, "/opt/trn_rl_repo")

import numpy as np
import ml_dtypes

import concourse.bass as bass
import concourse.mybir as mybir
import concourse.tile as tile
from concourse import bacc
from concourse import bass_utils

dt = mybir.dt
AF = mybir.ActivationFunctionType
ALU = mybir.AluOpType
bf16 = ml_dtypes.bfloat16
f8 = ml_dtypes.float8_e4m3

NCORES = 8
D = 128
NL_TOT, NCL_TOT, NV = 8192, 16384, 4096
NL = NL_TOT // NCORES      # 1024 lits per core
NCL = NCL_TOT // NCORES    # 2048 clauses per core
KT1 = NL_TOT // 128        # 64  k-tiles for A^T @ Lpre
KT2 = NCL_TOT // 128       # 128 k-tiles for A @ Cpre
FP8_ONE = 0x38             # bit pattern of 1.0 in float8_e4m3
GAIN = np.float32(128.0)   # power-of-2 pre-scale keeping fp8 messages normal-range


# ---------------------------------------------------------------------------
# device program
# ---------------------------------------------------------------------------

def build_program(rounds: int):
    nc = bacc.Bacc("TRN2", target_bir_lowering=False, debug=False,
                   num_devices=NCORES)

    def inp(name, shape, dty):
        return nc.dram_tensor(name, list(shape), dty, kind="ExternalInput")

    # B streams, pre-tiled to match the chunk-major contraction loops:
    # b1[nn, h]: slab of 32 k-tiles [128p, 32tt, 512c] covering clause chunk
    #            nn, lit-tile phase h (tt = 4k + jj, global tile t = 8k+4h+jj)
    # b2[nn, h, s]: slab of 32 k-tiles [128p, 32tt, 512l] covering lit chunk
    #            nn, clause-tile phase h, half s (tt=8kk+j2, T=16(4s+kk)+8h+j2)
    b1 = inp("b1", [4, 2, 128, 32 * 512], dt.float8e4)
    b2 = inp("b2", [2, 2, 2, 128, 32 * 512], dt.float8e4)
    w = {}
    for p in ("lm", "cm", "lv"):
        for l in ("w1t", "w2t", "w3t"):
            shape = [128, 1] if (p, l) == ("lv", "w3t") else [128, 128]
            w[f"{p}_{l}"] = inp(f"{p}_{l}", shape, dt.bfloat16)
        for l in ("b1", "b2"):
            w[f"{p}_{l}"] = inp(f"{p}_{l}", [128, 1], dt.float32)
    cu_wt = inp("cu_wt", [128, 512], dt.bfloat16)      # cu_wih.T
    cu_ut = inp("cu_ut", [128, 512], dt.bfloat16)      # cu_whh.T
    cu_b = inp("cu_b", [128, 4], dt.float32)
    lu_wcl = inp("lu_wcl", [128, 512], dt.bfloat16)    # lu_wih[:, :128].T
    lu_wfl = inp("lu_wfl", [128, 512], dt.bfloat16)    # lu_wih[:, 128:].T
    lu_ut = inp("lu_ut", [128, 512], dt.bfloat16)      # lu_whh.T
    lu_b = inp("lu_b", [128, 4], dt.float32)
    lm_b3r = inp("lm_b3r", [1, 512], dt.bfloat16)      # lm_b3 tiled 4x
    cm_b3r = inp("cm_b3r", [1, 512], dt.bfloat16)      # cm_b3 tiled 4x
    ones1 = inp("ones1", [1, 128], dt.bfloat16)
    # round-0 shortcut: Lh0 is a broadcast vector, so round 0's dir-1 output
    # is rank-1: ps1 = u0 (x) sb1r with u0 = mlp(lh0)+b3 (host, exact) and
    # sb1r = GAIN * (B^T row) over my clauses
    u0 = inp("u0", [1, 128], dt.bfloat16)
    sb1r = inp("sb1r", [1, NCL], dt.bfloat16)
    colb = inp("colb", [128, NCL], dt.bfloat16)         # col bcast over partitions
    rowb = inp("rowb", [128, NL], dt.bfloat16)          # row bcast over partitions
    rowsc = inp("rowsc", [128, 8], dt.float32)         # row, per lit-tile column
    colsc = inp("colsc", [128, 16], dt.float32)        # col, per clause-tile column
    lh0 = inp("lh0", [128, NL], dt.bfloat16)
    ch0 = inp("ch0", [128, NCL], dt.bfloat16)

    vote_out = nc.dram_tensor("vote", [1, NL], dt.float32, kind="ExternalOutput")
    # per-phase AllGather buffers: AG1a/b carry lit chunks 0/1 (the tiles
    # dir-1 phase h=0/1 contracts); AG2a/b carry clause chunks {0,1}/{2,3}
    # (what dir-2 phase h=0/1 contracts).
    ag1_out = [nc.dram_tensor(f"ag1{h}_out", [NCORES, 128, 512], dt.float8e4,
                              addr_space="Shared") for h in range(2)]
    ag2_out = [nc.dram_tensor(f"ag2{q}_out", [NCORES, 128, 512], dt.float8e4,
                              addr_space="Shared") for q in range(4)]
    rg = [list(range(NCORES))]

    with tile.TileContext(nc) as tc:
        with (
            tc.tile_pool(name="const", bufs=1) as cp,
            tc.tile_pool(name="state", bufs=1) as sp,
            tc.tile_pool(name="work", bufs=1) as wp,
            tc.tile_pool(name="chunk", bufs=2) as kp,
            tc.tile_pool(name="bstream", bufs=4) as bp,
            tc.tile_pool(name="psd", bufs=1, space="PSUM") as psd,
            tc.tile_pool(name="psg", bufs=1, space="PSUM") as psg,
            tc.tile_pool(name="psm", bufs=2, space="PSUM") as psm,
            tc.tile_pool(name="dram", bufs=1, space="DRAM") as dp,
        ):
            # ---- constants into SBUF ----
            C = {}
            for name, t in [
                ("cu_wt", cu_wt), ("cu_ut", cu_ut), ("lu_wcl", lu_wcl),
                ("lu_wfl", lu_wfl), ("lu_ut", lu_ut),
            ]:
                C[name] = cp.tile([128, 512], dt.bfloat16, name=name)
                nc.scalar.dma_start(out=C[name], in_=t.ap())
            for p in ("lm", "cm", "lv"):
                for l in ("w1t", "w2t", "w3t"):
                    shape = [128, 1] if (p, l) == ("lv", "w3t") else [128, 128]
                    C[f"{p}_{l}"] = cp.tile(shape, dt.bfloat16, name=f"{p}_{l}")
                    nc.scalar.dma_start(out=C[f"{p}_{l}"], in_=w[f"{p}_{l}"].ap())
                for l in ("b1", "b2"):
                    C[f"{p}_{l}"] = cp.tile([128, 1], dt.float32, name=f"{p}_{l}")
                    nc.scalar.dma_start(out=C[f"{p}_{l}"], in_=w[f"{p}_{l}"].ap())
            for name, t, shape, dty in [
                ("cu_b", cu_b, [128, 4], dt.float32),
                ("lu_b", lu_b, [128, 4], dt.float32),
                ("lm_b3r", lm_b3r, [1, 512], dt.bfloat16),
                ("cm_b3r", cm_b3r, [1, 512], dt.bfloat16),
                ("ones1", ones1, [1, 128], dt.bfloat16),
                ("u0", u0, [1, 128], dt.bfloat16),
                ("sb1r", sb1r, [1, NCL], dt.bfloat16),
                ("colb", colb, [128, NCL], dt.bfloat16),
                ("rowb", rowb, [128, NL], dt.bfloat16),
                ("rowsc", rowsc, [128, 8], dt.float32),
                ("colsc", colsc, [128, 16], dt.float32),
            ]:
                C[name] = cp.tile(shape, dty, name=name)
                nc.scalar.dma_start(out=C[name], in_=t.ap())

            # ---- states ----
            Lh_pp = [sp.tile([128, NL], dt.bfloat16, name="Lh_a"),
                     sp.tile([128, NL], dt.bfloat16, name="Lh_b")]
            Ch = sp.tile([128, NCL], dt.bfloat16, name="Ch")
            Lc = sp.tile([128, NL], dt.bfloat16, name="Lc")
            Cc = sp.tile([128, NCL], dt.bfloat16, name="Cc")
            nc.scalar.dma_start(out=Lh_pp[0], in_=lh0.ap())
            nc.scalar.dma_start(out=Ch, in_=ch0.ap())
            nc.vector.memset(Lc, 0.0)
            nc.vector.memset(Cc, 0.0)

            # ---- resident slices of B: clause-chunk 0 of b1 (both phases)
            # plus a few pinned b2 slab-halves (cuts per-round streaming) ----
            b1res = [cp.tile([128, 32 * 512], dt.float8e4, name=f"b1res{h}")
                     for h in range(2)]
            for h in range(2):
                nc.sync.dma_start(out=b1res[h], in_=b1.ap()[0, h])
            PIN = [(0, 0, 0, 0), (0, 1, 0, 0), (1, 0, 0, 0), (1, 1, 0, 0), (0, 0, 1, 0)]
            b2res = {}
            for (pn, ph, psl, ps2) in PIN:
                tpin = cp.tile([128, 16 * 512], dt.float8e4,
                               name=f"b2res{pn}{ph}{psl}{ps2}")
                nc.sync.dma_start(
                    out=tpin,
                    in_=b2.ap()[pn, ph, psl][:, 16 * 512 * ps2:
                                             16 * 512 * (ps2 + 1)])
                b2res[(pn, ph, psl, ps2)] = tpin

            # ---- round-persistent work tiles ----
            lpre_img = wp.tile([128, NL], dt.float8e4, name="lpre_img")
            cpre_img = wp.tile([128, NCL], dt.float8e4, name="cpre_img")
            # gathered message halves: lpre_half[h] holds, for every core kk,
            # its chunk-h message tiles (4 tiles of 128 lits each) at columns
            # [512*kk, 512*kk+512); cpre_half[h] likewise with 8 tiles of 128
            # clauses at [1024*kk, 1024*kk+1024).
            lpre_half = [wp.tile([128, 8 * 512], dt.float8e4, name=f"lpre_h{h}")
                         for h in range(2)]
            cpre_half = [wp.tile([128, 8 * 1024], dt.float8e4, name=f"cpre_h{h}")
                         for h in range(2)]
            ag1_in = [dp.tile([128, 512], dt.float8e4, name=f"ag1{h}_in")
                      for h in range(2)]
            ag2_in = [dp.tile([128, 512], dt.float8e4, name=f"ag2{q}_in")
                      for q in range(4)]

            def mlp3_chunk(src, w1t, b1_, w2t, b2_, w3t, b3r, img, nn, sc):
                """3-layer MLP on one 512-col chunk; the 3rd layer transposes
                tile-wise into `img` and folds b3 in as a rank-1 matmul so the
                fp8 messages carry the full affine output (no gate-side
                correction needed)."""
                tag_sfx = "m"
                h1 = kp.tile([128, 512], dt.bfloat16, tag="mh1", bufs=2,
                             name=f"mh1_{nn}")
                h2 = kp.tile([128, 512], dt.bfloat16, tag="mh2", bufs=2,
                             name=f"mh2_{nn}")
                ps = psm.tile([128, 512], dt.float32, tag=tag_sfx,
                              name=f"mm1_{nn}")
                nc.tensor.matmul(ps, w1t, src, start=True, stop=True)
                nc.scalar.activation(h1, ps, AF.Relu, bias=b1_)
                ps = psm.tile([128, 512], dt.float32, tag=tag_sfx,
                              name=f"mm2_{nn}")
                nc.tensor.matmul(ps, w2t, h1, start=True, stop=True)
                nc.scalar.activation(h2, ps, AF.Relu, bias=b2_)
                ps3 = psm.tile([128, 512], dt.float32, tag=tag_sfx,
                               name=f"mm3_{nn}")
                for jj in range(4):
                    nc.tensor.matmul(ps3[:, 128 * jj:128 * (jj + 1)],
                                     h2[:, 128 * jj:128 * (jj + 1)], w3t,
                                     start=(jj == 0), stop=False,
                                     skip_group_check=True)
                nc.tensor.matmul(ps3, C["ones1"], b3r,
                                 start=False, stop=True, skip_group_check=True)
                for jj in range(4):
                    j = 4 * nn + jj
                    nc.scalar.activation(img[:, 128 * j:128 * (j + 1)],
                                         ps3[:, 128 * jj:128 * (jj + 1)],
                                         AF.Copy, scale=sc[:, j:j + 1])

            def l_msg_chunk(Lh_src, nn):
                """L-message for lit chunk nn (512 lits) + AG1-half kickoff."""
                sl = slice(512 * nn, 512 * (nn + 1))
                mlp3_chunk(Lh_src[:, sl], C["lm_w1t"], C["lm_b1"],
                           C["lm_w2t"], C["lm_b2"], C["lm_w3t"], C["lm_b3r"],
                           lpre_img, nn, C["rowsc"])
                nc.gpsimd.dma_start(out=ag1_in[nn], in_=lpre_img[:, sl])
                nc.gpsimd.collective_compute(
                    "AllGather", ALU.bypass, replica_groups=rg,
                    ins=[ag1_in[nn].opt()], outs=[ag1_out[nn].ap().opt()])

            def land_ag1(h):
                for kk in range(0, NCORES, 2):
                    nc.gpsimd.dma_start(
                        out=lpre_half[h][:, 512 * kk:512 * (kk + 2)]
                        .rearrange("p (k c) -> p k c", k=2),
                        in_=ag1_out[h].ap()[kk:kk + 2]
                        .rearrange("k p c -> p k c"))

            def c_msg_chunk(nn):
                """C-message for clause chunk nn (512 clauses)."""
                sl = slice(512 * nn, 512 * (nn + 1))
                mlp3_chunk(Ch[:, sl], C["cm_w1t"], C["cm_b1"],
                           C["cm_w2t"], C["cm_b2"], C["cm_w3t"], C["cm_b3r"],
                           cpre_img, nn, C["colsc"])
                nc.gpsimd.dma_start(out=ag2_in[nn], in_=cpre_img[:, sl])
                nc.gpsimd.collective_compute(
                    "AllGather", ALU.bypass, replica_groups=rg,
                    ins=[ag2_in[nn].opt()], outs=[ag2_out[nn].ap().opt()])

            def land_ag2(h):
                # phase h consumes clause chunks {2h, 2h+1}: quarter-gather q
                # lands at columns 1024*kk + 512*(q%2); two cores per DMA via
                # a strided 4D view (halves the serial gpsimd issue time)
                for q in (2 * h, 2 * h + 1):
                    v = cpre_half[h].rearrange("p (a b c) -> p a b c",
                                               a=NCORES, b=2)
                    for kk in range(0, NCORES, 2):
                        nc.gpsimd.dma_start(
                            out=v[:, kk:kk + 2, q % 2, :],
                            in_=ag2_out[q].ap()[kk:kk + 2]
                            .rearrange("k p c -> p k c"))

            def lstm_chunk(which, cc, xin, Lh_src=None, Lh_dst=None):
                """LSTM gate + state update for one 512-col chunk."""
                sl = slice(512 * cc, 512 * (cc + 1))
                gts = []
                for g in range(4):
                    gs = slice(128 * g, 128 * (g + 1))
                    ps = psg.tile([128, 512], dt.float32, tag=f"g{g % 2}",
                                  name=f"ps_{which}_{cc}_{g}")
                    if which == "c":
                        nc.tensor.matmul(ps, C["cu_wt"][:, gs], xin,
                                         start=True, stop=False,
                                         skip_group_check=True)
                        nc.tensor.matmul(ps, C["cu_ut"][:, gs], Ch[:, sl],
                                         start=False, stop=True,
                                         skip_group_check=True)
                        bias = C["cu_b"][:, g:g + 1]
                    else:
                        flip_sl = slice(512 * (1 - cc), 512 * (2 - cc))
                        nc.tensor.matmul(ps, C["lu_wcl"][:, gs], xin,
                                         start=True, stop=False,
                                         skip_group_check=True)
                        nc.tensor.matmul(ps, C["lu_wfl"][:, gs],
                                         Lh_src[:, flip_sl],
                                         start=False, stop=False,
                                         skip_group_check=True)
                        nc.tensor.matmul(ps, C["lu_ut"][:, gs], Lh_src[:, sl],
                                         start=False, stop=True,
                                         skip_group_check=True)
                        bias = C["lu_b"][:, g:g + 1]
                    gt = kp.tile([128, 512], dt.bfloat16, tag=f"gate{g}",
                                 bufs=1, name=f"gt_{which}_{cc}_{g}")
                    nc.scalar.activation(gt, ps,
                                         AF.Tanh if g == 2 else AF.Sigmoid,
                                         bias=bias)
                    gts.append(gt)
                cell = Cc if which == "c" else Lc
                hout = Ch if which == "c" else Lh_dst
                t1 = kp.tile([128, 512], dt.bfloat16, tag="t1", bufs=1,
                             name=f"t1_{which}_{cc}")
                t2 = kp.tile([128, 512], dt.bfloat16, tag="t2", bufs=1,
                             name=f"t2_{which}_{cc}")
                nc.vector.tensor_tensor(out=t1, in0=gts[1], in1=cell[:, sl],
                                        op=ALU.mult)
                nc.vector.tensor_tensor(out=t2, in0=gts[0], in1=gts[2],
                                        op=ALU.mult)
                nc.vector.tensor_tensor(out=cell[:, sl], in0=t1, in1=t2,
                                        op=ALU.add)
                t3 = kp.tile([128, 512], dt.bfloat16, tag="t3", bufs=1,
                             name=f"t3_{which}_{cc}")
                nc.scalar.activation(t3, cell[:, sl], AF.Tanh)
                nc.vector.tensor_tensor(out=hout[:, sl], in0=gts[3], in1=t3,
                                        op=ALU.mult)

            # Manual phase pinning: monotonically increasing scheduler-sim
            # timestamps force the emitted per-engine instruction order to
            # follow the hand-pipelined phase order. Without this, the
            # scheduler's naive collective cost model emits AG-completion
            # waits (landing copies) ahead of the next collective's trigger
            # on the gpsimd queue, head-blocking it for ~15us per round.
            _ph = [0]

            def phase():
                _ph[0] += 1
                return tc.tile_wait_until(_ph[0])

            # (no prologue: round 0's dir-1 collapses to rank-1 matmuls, so
            # no round-0 L messages or AG1s are needed)

            for r in range(rounds):
                Lh = Lh_pp[r % 2]
                Lh_new = Lh_pp[(r + 1) % 2]

                # ===== dir-1, group-major: clause chunks {0,1} are fully
                # contracted (h=0 then h=1) and their C side run first, so
                # AG2a kicks at ~50% of the C-phase with the whole second
                # group as its in-flight cover; group {2,3} then feeds AG2b,
                # which flies over dir-2 h=0 =====
                ps1 = [psd.tile([128, 512], dt.float32, tag=f"d{nn}",
                                name=f"ps1_{r}_{nn}") for nn in range(4)]

                def d1_contract(nn, h):
                    for s2 in range(2):
                        if nn == 0:
                            b1t = b1res[h][:, 16 * 512 * s2:
                                           16 * 512 * (s2 + 1)]
                        else:
                            b1t = bp.tile([128, 16 * 512], dt.float8e4,
                                          tag="b1", bufs=3,
                                          name=f"b1_{r}_{nn}_{h}_{s2}")
                            nc.sync.dma_start(
                                out=b1t,
                                in_=b1.ap()[nn, h][:, 16 * 512 * s2:
                                                   16 * 512 * (s2 + 1)])
                        for ttp in range(8):
                            tt = 16 * s2 + 2 * ttp
                            lhsT = lpre_half[h][:, 128 * tt:128 * (tt + 2)] \
                                .rearrange("p (e d) -> p e d", e=2)
                            rhs = b1t[:, 1024 * ttp:1024 * (ttp + 1)] \
                                .rearrange("p (e c) -> p e c", e=2)
                            nc.tensor.matmul(
                                ps1[nn], lhsT, rhs,
                                start=(h == 0 and tt == 0),
                                stop=(h == 1 and tt == 30),
                                perf_mode=mybir.MatmulPerfMode.DoubleRow,
                                skip_group_check=True)

                def c_xin(cn):
                    # hoisted ahead of the LSTM chains so the vector queue
                    # never head-blocks a gate matmul on a late xin
                    sl = slice(512 * cn, 512 * (cn + 1))
                    xin = kp.tile([128, 512], dt.bfloat16, tag="xin",
                                  bufs=2, name=f"lcs_{r}_{cn}")
                    nc.vector.tensor_tensor(out=xin, in0=ps1[cn],
                                            in1=C["colb"][:, sl],
                                            op=ALU.mult)
                    return xin

                def c_rest(cn, xin):
                    lstm_chunk("c", cn, xin)
                    c_msg_chunk(cn)

                for g in range(2):
                    n0, n1 = 2 * g, 2 * g + 1
                    if r == 0:
                        with phase():
                            for cn in (n0, n1):
                                sl = slice(512 * cn, 512 * (cn + 1))
                                nc.tensor.matmul(ps1[cn], C["u0"],
                                                 C["sb1r"][0:1, sl],
                                                 start=True, stop=True,
                                                 skip_group_check=True)
                            x0 = c_xin(n0)
                            x1 = c_xin(n1)
                            c_rest(n0, x0)
                            c_rest(n1, x1)      # kicks AG2a / AG2b
                    elif g == 0:
                        with phase():
                            land_ag1(0)
                        with phase():
                            d1_contract(n0, 0)
                        with phase():
                            land_ag1(1)
                        with phase():
                            d1_contract(n0, 1)
                            x0 = c_xin(n0)
                            d1_contract(n1, 0)
                            c_rest(n0, x0)      # kicks AG2-q0 early
                            d1_contract(n1, 1)
                            x1 = c_xin(n1)
                            c_rest(n1, x1)      # kicks AG2-q1
                    else:
                        with phase():
                            d1_contract(n0, 0)
                            d1_contract(n0, 1)
                            x0 = c_xin(n0)
                            d1_contract(n1, 0)
                            c_rest(n0, x0)      # kicks AG2-q2 early
                            d1_contract(n1, 1)
                            x1 = c_xin(n1)
                            c_rest(n1, x1)      # kicks AG2-q3

                # ===== dir-2, phase-major: h=0 contracts the AG2a tiles for
                # both lit chunks while AG2b flies; h=1 completes each chunk
                # and runs the L side, kicking the next round's AG1 halves =====
                ps2 = [psd.tile([128, 512], dt.float32, tag=f"d{nn}",
                                name=f"ps2_{r}_{nn}") for nn in range(2)]
                lxin = [None, None]
                for h in range(2):
                    with phase():
                        land_ag2(h)
                    with phase():
                        for nn in range(3 if h == 1 else 2):
                            if nn < 2:
                                for s in range(2):
                                    for s2 in range(2):
                                        if (nn, h, s, s2) in b2res:
                                            b2t = b2res[(nn, h, s, s2)]
                                        else:
                                            b2t = bp.tile([128, 16 * 512],
                                                          dt.float8e4,
                                                          tag="b2", bufs=4,
                                                          name=f"b2_{r}_{nn}_{h}_{s}_{s2}")
                                            nc.sync.dma_start(
                                                out=b2t,
                                                in_=b2.ap()[nn, h, s][:, 16 * 512 * s2:
                                                                      16 * 512 * (s2 + 1)])
                                        for ttp in range(8):
                                            tt = 16 * s2 + 2 * ttp
                                            lhsT = cpre_half[h][:, 4096 * s + 128 * tt:
                                                                4096 * s + 128 * (tt + 2)] \
                                                .rearrange("p (e d) -> p e d", e=2)
                                            rhs = b2t[:, 1024 * ttp:1024 * (ttp + 1)] \
                                                .rearrange("p (e c) -> p e c", e=2)
                                            nc.tensor.matmul(
                                                ps2[nn], lhsT, rhs,
                                                start=(h == 0 and s == 0 and tt == 0),
                                                stop=(h == 1 and s == 1 and tt == 30),
                                                perf_mode=mybir.MatmulPerfMode.DoubleRow,
                                                skip_group_check=True)
                            if h == 1 and nn < 2:
                                sl = slice(512 * nn, 512 * (nn + 1))
                                lxin[nn] = kp.tile([128, 512], dt.bfloat16,
                                                   tag="xin", bufs=2,
                                                   name=f"cls_{r}_{nn}")
                                nc.vector.tensor_tensor(out=lxin[nn],
                                                        in0=ps2[nn],
                                                        in1=C["rowb"][:, sl],
                                                        op=ALU.mult)
                            if h == 1 and nn > 0:
                                cn = nn - 1
                                lstm_chunk("l", cn, lxin[cn], Lh_src=Lh,
                                           Lh_dst=Lh_new)
                                if r < rounds - 1:
                                    l_msg_chunk(Lh_new, cn)

            # ===== vote MLP (bias of last layer added host-side) =====
            Lh_fin = Lh_pp[rounds % 2]
            vote_sb = wp.tile([1, NL], dt.float32, name="vote_sb")
            for nn in range(2):
                sl = slice(512 * nn, 512 * (nn + 1))
                vh1 = kp.tile([128, 512], dt.bfloat16, tag="mh1", bufs=2,
                              name=f"vh1_{nn}")
                vh2 = kp.tile([128, 512], dt.bfloat16, tag="mh2", bufs=2,
                              name=f"vh2_{nn}")
                ps = psm.tile([128, 512], dt.float32, tag="m", name=f"v1_{nn}")
                nc.tensor.matmul(ps, C["lv_w1t"], Lh_fin[:, sl],
                                 start=True, stop=True)
                nc.scalar.activation(vh1, ps, AF.Relu, bias=C["lv_b1"])
                ps = psm.tile([128, 512], dt.float32, tag="m", name=f"v2_{nn}")
                nc.tensor.matmul(ps, C["lv_w2t"], vh1,
                                 start=True, stop=True)
                nc.scalar.activation(vh2, ps, AF.Relu, bias=C["lv_b2"])
                ps = psm.tile([1, 512], dt.float32, tag="m", name=f"v3_{nn}")
                nc.tensor.matmul(ps, C["lv_w3t"], vh2,
                                 start=True, stop=True)
                nc.scalar.activation(vote_sb[0:1, sl], ps, AF.Copy)
            nc.scalar.dma_start(out=vote_out.ap(), in_=vote_sb)

    nc.compile()
    return nc


# ---------------------------------------------------------------------------
# host-side input preparation
# ---------------------------------------------------------------------------

def prep_inputs(inputs):
    g = {k: np.asarray(v) for k, v in inputs.items()}
    lit_idx = g["lit_idx"].astype(np.int64)
    clause_idx = g["clause_idx"].astype(np.int64)

    B = np.zeros((NL_TOT, NCL_TOT), np.bool_)
    B[lit_idx, clause_idx] = True
    degc = B.sum(0).astype(np.float32)
    degl = B.sum(1).astype(np.float32)
    col = (np.float32(1.0) / (np.sqrt(degc) + np.float32(1e-6))).astype(np.float32)
    row = (np.float32(1.0) / (np.sqrt(degl) + np.float32(1e-6))).astype(np.float32)
    # degree-0 rows/cols of A are structurally zero: clamp their scales so the
    # gained fp8 messages stay finite (mathematically identical result)
    col = np.where(degc > 0, col, np.float32(0)).astype(np.float32)
    row = np.where(degl > 0, row, np.float32(0)).astype(np.float32)

    # permuted lit order: core k <- [512k..512k+512) u [4096+512k..4096+512k+512)
    lit_order = np.concatenate(
        [np.concatenate([np.arange(512 * k, 512 * (k + 1)),
                         NV + np.arange(512 * k, 512 * (k + 1))])
         for k in range(NCORES)])
    Bu = B.astype(np.uint8) * FP8_ONE
    Bp = Bu[lit_order]                      # [8192, 16384] permuted rows
    row_p = row[lit_order]

    def b(x):
        return np.ascontiguousarray(np.asarray(x, np.float32)).astype(bf16)

    # round-0 shortcut constants: u0 = lm_mlp(lh0_vec)+lm_b3 (exact, host),
    # sb1 = GAIN * (row @ B) per clause
    lh0_vec = np.asarray(g["L_init_w"], np.float32)[:, 0] + np.asarray(
        g["L_init_b"], np.float32)
    _h = np.maximum(np.asarray(g["lm_w1"], np.float32) @ lh0_vec
                    + np.asarray(g["lm_b1"], np.float32), 0)
    _h = np.maximum(np.asarray(g["lm_w2"], np.float32) @ _h
                    + np.asarray(g["lm_b2"], np.float32), 0)
    u0_vec = np.asarray(g["lm_w3"], np.float32) @ _h + np.asarray(
        g["lm_b3"], np.float32)
    sb1_full = GAIN * (row @ B.astype(np.float32))

    common = {
        "lm_w1t": b(g["lm_w1"].T), "lm_w2t": b(g["lm_w2"].T), "lm_w3t": b(g["lm_w3"].T),
        "cm_w1t": b(g["cm_w1"].T), "cm_w2t": b(g["cm_w2"].T), "cm_w3t": b(g["cm_w3"].T),
        "lv_w1t": b(g["lv_w1"].T), "lv_w2t": b(g["lv_w2"].T), "lv_w3t": b(g["lv_w3"].T),
        "lm_b1": np.asarray(g["lm_b1"], np.float32).reshape(128, 1),
        "lm_b2": np.asarray(g["lm_b2"], np.float32).reshape(128, 1),
        "cm_b1": np.asarray(g["cm_b1"], np.float32).reshape(128, 1),
        "cm_b2": np.asarray(g["cm_b2"], np.float32).reshape(128, 1),
        "lv_b1": np.asarray(g["lv_b1"], np.float32).reshape(128, 1),
        "lv_b2": np.asarray(g["lv_b2"], np.float32).reshape(128, 1),
        "cu_wt": b(g["cu_wih"].T), "cu_ut": b(g["cu_whh"].T),
        "lu_wcl": b(g["lu_wih"][:, :D].T), "lu_wfl": b(g["lu_wih"][:, D:].T),
        "lu_ut": b(g["lu_whh"].T),
        "cu_b": np.asarray(g["cu_bih"] + g["cu_bhh"], np.float32).reshape(4, 128).T.copy(),
        "lu_b": np.asarray(g["lu_bih"] + g["lu_bhh"], np.float32).reshape(4, 128).T.copy(),
        "lm_b3r": b(np.tile(np.asarray(g["lm_b3"], np.float32), 4)).reshape(1, 512),
        "cm_b3r": b(np.tile(np.asarray(g["cm_b3"], np.float32), 4)).reshape(1, 512),
        "ones1": np.ones((1, 128), dtype=bf16),
        "u0": b(u0_vec).reshape(1, 128),
        "lh0": np.ascontiguousarray(np.broadcast_to(
            np.asarray(g["L_init_w"][:, 0] + g["L_init_b"], np.float32)[:, None],
            (128, NL))).astype(bf16),
        "ch0": np.ascontiguousarray(np.broadcast_to(
            np.asarray(g["C_init_w"][:, 0] + g["C_init_b"], np.float32)[:, None],
            (128, NCL))).astype(bf16),
    }

    in_maps = []
    for k in range(NCORES):
        lsl = slice(NL * k, NL * (k + 1))
        csl = slice(NCL * k, NCL * (k + 1))
        # b1: Bp[:, csl] is [t(64)*128p rows, nn(4)*512c cols]
        #     [kk(8), jh(2), jj(4), p, nn, c] -> [nn, jh, p, kk, jj, c]
        X = Bp[:, csl].reshape(8, 2, 4, 128, 4, 512)
        b1k = np.ascontiguousarray(X.transpose(4, 1, 3, 0, 2, 5)).reshape(
            4, 2, 128, 32 * 512).view(f8)
        # b2: Bp[lsl, :].T is [T(128)*128p rows, nn(2)*512l cols]
        #     [ks(2), kk(4), h(2), j2(8), p, nn, l] -> [nn, h, ks, p, kk, j2, l]
        Y = Bp[lsl, :].T.reshape(2, 4, 2, 8, 128, 2, 512)
        b2k = np.ascontiguousarray(Y.transpose(5, 2, 0, 4, 1, 3, 6)).reshape(
            2, 2, 2, 128, 32 * 512).view(f8)
        m = dict(common)
        m.update({
            "b1": b1k,
            "b2": b2k,
            "sb1r": sb1_full[csl].astype(bf16).reshape(1, NCL),
            "colb": np.ascontiguousarray(
                np.broadcast_to(col[csl][None, :] / GAIN, (128, NCL))).astype(bf16),
            "rowb": np.ascontiguousarray(
                np.broadcast_to(row_p[lsl][None, :] / GAIN, (128, NL))).astype(bf16),
            "rowsc": np.ascontiguousarray(
                GAIN * row_p[lsl].reshape(8, 128).T).astype(np.float32),
            "colsc": np.ascontiguousarray(
                GAIN * col[csl].reshape(16, 128).T).astype(np.float32),
        })
        in_maps.append(m)
    return in_maps


def selfcheck_layouts(in_maps, lit_idx, clause_idx):
    """Random probes: device-layout b1/b2 entries vs the raw B matrix."""
    B = np.zeros((NL_TOT, NCL_TOT), np.uint8)
    B[lit_idx, clause_idx] = FP8_ONE
    lit_order = np.concatenate(
        [np.concatenate([np.arange(512 * k, 512 * (k + 1)),
                         NV + np.arange(512 * k, 512 * (k + 1))])
         for k in range(NCORES)])
    Bp = B[lit_order]
    rng = np.random.default_rng(1)
    for k in (0, 3):
        b1k = in_maps[k]["b1"].view(np.uint8).reshape(4, 2, 128, 32, 512)
        for _ in range(50):
            nn, h, p, tt, c = (rng.integers(4), rng.integers(2), rng.integers(128),
                               rng.integers(32), rng.integers(512))
            t = 8 * (tt // 4) + 4 * h + tt % 4
            want = Bp[128 * t + p, 2048 * k + 512 * nn + c]
            assert b1k[nn, h, p, tt, c] == want, (k, nn, h, p, tt, c)
        b2k = in_maps[k]["b2"].view(np.uint8).reshape(2, 2, 2, 128, 32, 512)
        for _ in range(50):
            nn, h, s, p, tt, c = (rng.integers(2), rng.integers(2), rng.integers(2),
                                  rng.integers(128), rng.integers(32), rng.integers(512))
            T = 16 * (4 * s + tt // 8) + 8 * h + tt % 8
            want = Bp[1024 * k + 512 * nn + c, 128 * T + p]
            assert b2k[nn, h, s, p, tt, c] == want, (k, nn, h, s, p, tt, c)


_PROGRAM_CACHE = {}


def _get_program(rounds):
    if rounds not in _PROGRAM_CACHE:
        _PROGRAM_CACHE[rounds] = build_program(rounds)
    return _PROGRAM_CACHE[rounds]


def run_device(inputs, trace=False, rounds=None, **kw):
    if rounds is None:
        rounds = int(inputs.get("n_rounds", 16))
    in_maps = prep_inputs(inputs)
    nc = _get_program(rounds)
    res = bass_utils.run_bass_kernel_spmd(
        nc, in_maps, core_ids=list(range(NCORES)), trace=trace, **kw)
    return res


def assemble_votes(res_results, lv_b3):
    votes = np.stack([np.asarray(res_results[k]["vote"]).reshape(NL)
                      for k in range(NCORES)])   # [8, 1024]
    vote = votes + np.float32(lv_b3)
    pos = vote[:, :512].reshape(NV)              # var v -> core v//512
    neg = vote[:, 512:].reshape(NV)
    vj = np.stack([pos, neg], axis=1)            # [4096, 2]
    return vj.reshape(32, -1).mean(axis=1).astype(np.float32)


def kernel(**inputs) -> np.ndarray:
    res = run_device(inputs)
    return assemble_votes(res.results, np.asarray(inputs["lv_b3"]).reshape(-1)[0])

